# revision 13
# baseline (speedup 1.0000x reference)
"""GraphSAGE layer on 8 Trainium2 NeuronCores (Bass/Tile).

Strategy: data-parallel over the 50000 target nodes (6250 rows/core),
feature table replicated per core as a bf16 copy with zero-row separators
between 7 chunks of <=28572 rows (so chunk-local row ids fit int16).
Neighbor + self rows are gathered with batched SWDGE dma_gather (one
instruction per (group, chunk, slot-batch), ~0.34ns/descriptor) instead of
per-128-row INDIRECT1D (~1.1us each). Per 4-tile group, each chunk owns a
quota of gather slots; unused positions carry idx=-1, which the ucode
resolves to (chunk base - 1 row) = a zero separator row, so dummy slots
contribute exact zeros to the slot-fold tree. Targets are greedily
grouped to minimize per-group chunk quotas. Per tile: bf16 in-place fold
tree over slots -> total sum; small tree over the 7 self slots -> self
row; agg = total - self. Then (as before) PE transposes + matmuls
(out.T = W1 @ self.T + (W2/25) @ agg.T), ReLU+bias with fused BN-stat
accumulation, AllReduce of (sum, sumsq), BN apply + row L2-normalize,
per-shard output written back and un-permuted on host.
"""
from contextlib import ExitStack

import numpy as np
import ml_dtypes

import concourse.bacc as bacc
import concourse.bass as bass
import concourse.tile as tile
from concourse import mybir
from concourse.bass_utils import run_bass_kernel_spmd
from concourse.masks import make_identity

BN_EPS = 1e-5
NORM_EPS = 1e-6

N_CORES = 8
N_TOTAL = 50000
TABLE_ROWS = 200000
D = 128
S_NEIGH = 25
P = 128
G = 4                    # tiles per group
K_CHUNK = 7
CH = 28572               # chunk rows (< 32768 for int16)
MAX_SLOT_BATCH = 14      # slots per gather instruction (num_idxs <= 7168)
S_CAP = 60               # max slots per group (SBUF: gbuf = S*G*256B/part)

_prog_cache = {}


# --------------------------------------------------------------------------
# Host-side layout construction
# --------------------------------------------------------------------------

def _greedy_group_joint(counts_all, caps):
    """Jointly assign each core's targets to groups against SHARED quota
    vectors, minimizing sum_c max(count_c) per group (max taken over the
    union of all cores' members of that group).

    counts_all: [n_cores, n, K]. caps: per-group per-core capacity.
    Returns assign[n_cores, n], pos[n_cores, n], gmax[ngroups, K].
    """
    n_cores, n, _ = counts_all.shape
    ngroups = len(caps)
    caps = np.asarray(caps, dtype=np.int64)
    gmax = np.zeros((ngroups, K_CHUNK), dtype=np.int64)
    fill = np.zeros((n_cores, ngroups), dtype=np.int64)
    assign = np.empty((n_cores, n), dtype=np.int64)
    pos = np.empty((n_cores, n), dtype=np.int64)
    slot_budget = S_CAP - K_CHUNK     # neighbor slots only (self = +7)
    big = np.iinfo(np.int64).max
    # interleave cores, targets in descending max-count order per core
    orders = [np.argsort(-counts_all[k].max(axis=1), kind="stable")
              for k in range(n_cores)]
    for j in range(n):
        for k in range(n_cores):
            i = orders[k][j]
            newmax = np.maximum(gmax, counts_all[k][i])
            delta = newmax.sum(axis=1) - gmax.sum(axis=1)
            bad = ((fill[k] >= caps)
                   | (newmax.sum(axis=1) > slot_budget))
            if bad.all():
                delta = np.where(fill[k] < caps, delta, big)
            else:
                delta = np.where(bad, big, delta)
            g = int(np.argmin(delta))
            assign[k][i] = g
            pos[k][i] = fill[k][g]
            fill[k][g] += 1
            gmax[g] = np.maximum(gmax[g], counts_all[k][i])
    return assign, pos, gmax


def _build_layout(self_idx, neigh_idx):
    """Compute the shared (across cores) slot layout + per-core idx slabs.

    Returns (layout, idx_slabs, perms):
      layout: dict with n_groups, quotas n[g][c], slot starts a[g][c], S[g],
        per-instruction list per group: (dst_slot0, nslots, col0, npos,
        nvalid), total idx columns.
      idx_slabs: [n_cores] arrays [128, totcols] int16.
      perms: [n_cores] arrays row_of_target[i] (row index in the core's
        out tensor for local target i).
    """
    n = self_idx.shape[0]
    rpc = n // N_CORES
    n_tiles = (rpc + P - 1) // P
    n_groups = (n_tiles + G - 1) // G
    caps = [min(G * P, max(0, rpc - g * G * P)) for g in range(n_groups)]

    nb_all = np.empty((N_CORES, rpc, S_NEIGH), dtype=np.int64)
    sf_all = np.empty((N_CORES, rpc), dtype=np.int64)
    counts_all = np.empty((N_CORES, rpc, K_CHUNK), dtype=np.int64)
    for k in range(N_CORES):
        sl = slice(k * rpc, (k + 1) * rpc)
        nb_all[k] = np.asarray(neigh_idx[sl], dtype=np.int64)
        sf_all[k] = np.asarray(self_idx[sl], dtype=np.int64)
        ch = nb_all[k] // CH
        counts_all[k] = (
            ch[:, :, None] == np.arange(K_CHUNK)[None, None, :]).sum(1)
    assign_all, pos_all, nq = _greedy_group_joint(counts_all, caps)
    per_core = [(nb_all[k], sf_all[k], assign_all[k], pos_all[k])
                for k in range(N_CORES)]

    # slot layout per group: chunk c -> [a_c, a_c + n_c) neighbors,
    # a_c + n_c = self slot. S_g = sum (n_c + 1).
    a = np.zeros((n_groups, K_CHUNK), dtype=np.int64)
    S = np.zeros(n_groups, dtype=np.int64)
    for g in range(n_groups):
        acc = 0
        for c in range(K_CHUNK):
            a[g, c] = acc
            acc += nq[g, c] + 1
        S[g] = acc

    # per-group instruction list (split big chunks into slot batches)
    instrs = []           # per group: list of (c, slot0, nslots, col0, npos)
    gcol0 = []            # first column of each group's slab region
    col = 0
    for g in range(n_groups):
        gi = []
        gcol0.append(col)
        for c in range(K_CHUNK):
            total_slots = nq[g, c] + 1
            s0 = a[g, c]
            while total_slots > 0:
                k = min(MAX_SLOT_BATCH, total_slots)
                npos = k * G * P
                gi.append((c, int(s0), int(k), int(col), int(npos)))
                col += npos // 16
                s0 += k
                total_slots -= k
        instrs.append(gi)
    totcols = col

    # per-core slabs + valid counts
    slabs = np.full((N_CORES, 16, totcols), -1, dtype=np.int16)
    valid = np.zeros((N_CORES, sum(len(gi) for gi in instrs)), dtype=np.int64)
    perms = []
    ii_of = {}
    ii = 0
    for g in range(n_groups):
        for j, ins in enumerate(instrs[g]):
            ii_of[(g, j)] = ii
            ii += 1

    for k in range(N_CORES):
        nb, sf, assign, pos = per_core[k]
        row_of = np.empty(rpc, dtype=np.int64)
        # position array per group: A[s, t, p]
        for g in range(n_groups):
            m = np.where(assign == g)[0]
            A = np.full((int(S[g]), G, P), -1, dtype=np.int64)
            if len(m) > 0:
                t_of = pos[m] // P
                p_of = pos[m] % P
                row_of[m] = (g * G + t_of) * P + p_of
                srt = np.sort(nb[m], axis=1)
                chs = srt // CH
                loc = srt - chs * CH
                # rank within chunk: count of earlier entries in same chunk
                rank = (np.arange(S_NEIGH)[None, :]
                        - (srt[:, None, :]
                           < (chs * CH)[:, :, None]).sum(-1))
                slot = a[g][chs] + rank
                A[slot.reshape(-1),
                  np.repeat(t_of, S_NEIGH),
                  np.repeat(p_of, S_NEIGH)] = loc.reshape(-1)
                sc = sf[m] // CH
                A[(a[g] + nq[g])[sc], t_of, p_of] = sf[m] - sc * CH
            for j, (c, s0, nsl, col0, npos) in enumerate(instrs[g]):
                blk = A[s0:s0 + nsl].reshape(-1).copy()
                # all positions must be valid for multi-packet gathers:
                # dummies read the trailing zero-separator row (idx CH)
                blk[blk < 0] = CH
                valid[k, ii_of[(g, j)]] = npos
                slabs[k, :, col0:col0 + npos // 16] = (
                    blk.reshape(-1, 16).T.astype(np.int16))
        perms.append(row_of)

    vmax = valid.max(axis=0)

    layout = {
        "n_groups": n_groups,
        "n_tiles": n_tiles,
        "rpc": rpc,
        "S": tuple(int(x) for x in S),
        "instrs": tuple(tuple(gi) for gi in instrs),
        "gcol0": tuple(gcol0),
        "totcols": totcols,
        "vmax": tuple(int(x) for x in vmax),
        "ii_of": ii_of,
    }
    idx_slabs = [np.tile(slabs[k], (8, 1)) for k in range(N_CORES)]
    return layout, idx_slabs, perms


# --------------------------------------------------------------------------
# Device program
# --------------------------------------------------------------------------

def build_program(layout):
    n_groups = layout["n_groups"]
    n_tiles = layout["n_tiles"]
    rpc = layout["rpc"]
    S = layout["S"]
    instrs = layout["instrs"]
    gcol0 = layout["gcol0"]
    totcols = layout["totcols"]
    vmax = layout["vmax"]
    ii_of = layout["ii_of"]
    maxS = max(S)
    btab_rows = 1 + K_CHUNK * (CH + 1)

    nc = bacc.Bacc("TRN2", target_bir_lowering=False, num_devices=N_CORES)
    f32 = mybir.dt.float32
    bf16 = mybir.dt.bfloat16
    btab = nc.dram_tensor("btab", [btab_rows, D], bf16, kind="ExternalInput")
    idxslab = nc.dram_tensor("idxslab", [P, totcols], mybir.dt.int16,
                             kind="ExternalInput")
    w1t = nc.dram_tensor("w1t", [D, D], f32, kind="ExternalInput")
    w2ts = nc.dram_tensor("w2ts", [D, D], f32, kind="ExternalInput")
    bvec = nc.dram_tensor("bvec", [D, 1], f32, kind="ExternalInput")
    gvec = nc.dram_tensor("gvec", [D, 1], f32, kind="ExternalInput")
    betav = nc.dram_tensor("betav", [D, 1], f32, kind="ExternalInput")
    out = nc.dram_tensor("out", [rpc, D], f32, kind="ExternalOutput")

    ar_in = nc.dram_tensor("ar_in", [D, 2], f32)
    ar_out = nc.dram_tensor("ar_out", [D, 2], f32, addr_space="Shared")

    with tile.TileContext(nc) as tc:
        with ExitStack() as ctx:
            singles = ctx.enter_context(tc.tile_pool(name="singles", bufs=1))
            gpool = ctx.enter_context(tc.tile_pool(name="gpool", bufs=2))
            ipool = ctx.enter_context(tc.tile_pool(name="ipool", bufs=2))
            wpool = ctx.enter_context(tc.tile_pool(name="wpool", bufs=3))
            psum = ctx.enter_context(tc.tile_pool(name="psum", bufs=2,
                                                  space="PSUM"))
            psum2 = ctx.enter_context(tc.tile_pool(name="psum2", bufs=2,
                                                   space="PSUM"))

            w1t_sb = singles.tile([D, D], f32)
            nc.sync.dma_start(out=w1t_sb[:], in_=w1t[:])
            w2ts_sb = singles.tile([D, D], f32)
            nc.sync.dma_start(out=w2ts_sb[:], in_=w2ts[:])
            b_sb = singles.tile([D, 1], f32)
            nc.sync.dma_start(out=b_sb[:], in_=bvec[:])
            g_sb = singles.tile([D, 1], f32)
            nc.sync.dma_start(out=g_sb[:], in_=gvec[:])
            beta_sb = singles.tile([D, 1], f32)
            nc.sync.dma_start(out=beta_sb[:], in_=betav[:])
            ident = singles.tile([P, P], f32)
            make_identity(nc, ident[:])

            zbuf = singles.tile([P, n_tiles, P], f32)
            sums = singles.tile([P, n_tiles], f32)
            sumsq = singles.tile([P, n_tiles], f32)

            maxcols = max(
                sum(npos // 16 for (_, _, _, _, npos) in instrs[g])
                for g in range(n_groups))

            # ---------------- Phase A ------------------------------------
            for g in range(n_groups):
                Sg = S[g]
                cols_g = sum(npos // 16 for (_, _, _, _, npos) in instrs[g])
                idxt = ipool.tile([P, maxcols], mybir.dt.int16, tag="idxt")
                nc.sync.dma_start(
                    out=idxt[:, 0:cols_g],
                    in_=idxslab[:, gcol0[g]:gcol0[g] + cols_g])

                gbuf = gpool.tile([P, maxS * G, D], bf16, tag="gbuf")
                for j, (c, s0, nsl, col0, npos) in enumerate(instrs[g]):
                    base_c = 1 + c * (CH + 1)
                    lc0 = col0 - gcol0[g]
                    nc.gpsimd.dma_gather(
                        out_ap=gbuf[:, s0 * G:(s0 + nsl) * G, :],
                        in_ap=btab[base_c:base_c + CH + 1, :],
                        idxs_ap=idxt[:, lc0:lc0 + npos // 16],
                        num_idxs=npos,
                        num_idxs_reg=vmax[ii_of[(g, j)]],
                        elem_size=D,
                        single_packet=False)

                gview = gbuf[:].rearrange("p (s g) d -> p s g d", g=G)
                # self slot index per chunk = last slot of its region
                self_slots = []
                acc = 0
                for c in range(K_CHUNK):
                    nslc = sum(nsl for (cc, _, nsl, _, _) in instrs[g]
                               if cc == c)
                    self_slots.append(acc + nslc - 1)
                    acc += nslc

                for t in range(G):
                    tg = g * G + t
                    if tg >= n_tiles:
                        continue
                    nv = min(P, rpc - tg * P)

                    # self sum over the 7 self slots (bf16 -> f32 pairs)
                    st = wpool.tile([P, 4, D], f32, tag="st")
                    nc.vector.tensor_add(
                        st[:, 0, :],
                        gview[:, self_slots[0], t, :],
                        gview[:, self_slots[1], t, :])
                    nc.vector.tensor_add(
                        st[:, 1, :],
                        gview[:, self_slots[2], t, :],
                        gview[:, self_slots[3], t, :])
                    nc.vector.tensor_add(
                        st[:, 2, :],
                        gview[:, self_slots[4], t, :],
                        gview[:, self_slots[5], t, :])
                    nc.vector.tensor_copy(
                        out=st[:, 3, :], in_=gview[:, self_slots[6], t, :])
                    selfsum = wpool.tile([P, D], f32, tag="selfsum")
                    nc.vector.tensor_add(st[:, 0, :], st[:, 0, :],
                                         st[:, 1, :])
                    nc.vector.tensor_add(st[:, 2, :], st[:, 2, :],
                                         st[:, 3, :])
                    nc.vector.tensor_add(selfsum[:], st[:, 0, :],
                                         st[:, 2, :])

                    # in-place bf16 fold tree over all Sg slots
                    cur = Sg
                    while cur > 2:
                        if cur % 2 == 1:
                            nc.vector.tensor_add(
                                gview[:, 0, t, :],
                                gview[:, 0, t, :],
                                gview[:, cur - 1, t, :])
                            cur -= 1
                            if cur == 2:
                                break
                        h = cur // 2
                        nc.vector.tensor_add(
                            gview[:, 0:h, t, :],
                            gview[:, 0:h, t, :],
                            gview[:, h:cur, t, :])
                        cur = h
                    sall = wpool.tile([P, D], f32, tag="sall")
                    nc.vector.tensor_add(sall[:], gview[:, 0, t, :],
                                         gview[:, 1, t, :])
                    agg = wpool.tile([P, D], f32, tag="agg")
                    nc.vector.tensor_sub(agg[:], sall[:], selfsum[:])

                    # transposes via PE
                    pT = psum.tile([P, P], f32, tag="pT")
                    nc.tensor.transpose(out=pT[:], in_=selfsum[:],
                                        identity=ident[:])
                    sT = wpool.tile([P, P], f32, tag="sT")
                    nc.scalar.copy(out=sT[:], in_=pT[:])
                    pT2 = psum.tile([P, P], f32, tag="pT2")
                    nc.tensor.transpose(out=pT2[:], in_=agg[:],
                                        identity=ident[:])
                    aT = wpool.tile([P, P], f32, tag="aT")
                    nc.scalar.copy(out=aT[:], in_=pT2[:])

                    mm = psum2.tile([P, P], f32, tag="mm")
                    nc.tensor.matmul(mm[:], w1t_sb[:], sT[:],
                                     start=True, stop=False)
                    nc.tensor.matmul(mm[:], w2ts_sb[:], aT[:],
                                     start=False, stop=True)

                    if nv == P:
                        nc.scalar.activation(
                            out=zbuf[:, tg, :], in_=mm[:],
                            func=mybir.ActivationFunctionType.Relu,
                            bias=b_sb[:], scale=1.0,
                            accum_out=sums[:, tg:tg + 1])
                        dump = wpool.tile([P, P], f32, tag="dump")
                        nc.scalar.activation(
                            out=dump[:], in_=zbuf[:, tg, :],
                            func=mybir.ActivationFunctionType.Square,
                            accum_out=sumsq[:, tg:tg + 1])
                    else:
                        nc.scalar.activation(
                            out=zbuf[:, tg, 0:nv], in_=mm[:, 0:nv],
                            func=mybir.ActivationFunctionType.Relu,
                            bias=b_sb[:], scale=1.0,
                            accum_out=sums[:, tg:tg + 1])
                        dump = wpool.tile([P, P], f32, tag="dump")
                        nc.scalar.activation(
                            out=dump[:, 0:nv], in_=zbuf[:, tg, 0:nv],
                            func=mybir.ActivationFunctionType.Square,
                            accum_out=sumsq[:, tg:tg + 1])

            # ---------------- Phase B: global BN stats -------------------
            gstat = singles.tile([P, 2], f32)
            nc.vector.tensor_reduce(out=gstat[:, 0:1], in_=sums[:],
                                    axis=mybir.AxisListType.X,
                                    op=mybir.AluOpType.add)
            nc.vector.tensor_reduce(out=gstat[:, 1:2], in_=sumsq[:],
                                    axis=mybir.AxisListType.X,
                                    op=mybir.AluOpType.add)
            nc.sync.dma_start(out=ar_in[:], in_=gstat[:])
            nc.gpsimd.collective_compute(
                "AllReduce", mybir.AluOpType.add,
                ins=[ar_in[:]],
                outs=[ar_out[:]],
                replica_groups=[list(range(N_CORES))],
            )
            gg = singles.tile([P, 2], f32)
            nc.sync.dma_start(out=gg[:], in_=ar_out[:])

            inv_n = 1.0 / float(N_TOTAL)
            mu = singles.tile([P, 1], f32)
            nc.vector.tensor_scalar_mul(mu[:], gg[:, 0:1], inv_n)
            ex2 = singles.tile([P, 1], f32)
            nc.vector.tensor_scalar_mul(ex2[:], gg[:, 1:2], inv_n)
            var = singles.tile([P, 1], f32)
            nc.vector.tensor_mul(var[:], mu[:], mu[:])
            nc.vector.tensor_sub(var[:], ex2[:], var[:])
            nc.vector.tensor_scalar_add(var[:], var[:], BN_EPS)
            std = singles.tile([P, 1], f32)
            nc.scalar.sqrt(out=std[:], in_=var[:])
            rstd = singles.tile([P, 1], f32)
            nc.vector.reciprocal(out=rstd[:], in_=std[:])
            gp = singles.tile([P, 1], f32)
            nc.vector.tensor_mul(gp[:], g_sb[:], rstd[:])
            sh = singles.tile([P, 1], f32)
            nc.vector.tensor_mul(sh[:], mu[:], gp[:])
            nc.vector.tensor_sub(sh[:], beta_sb[:], sh[:])

            # ---------------- Phase C: BN apply + L2 normalize -----------
            for t in range(n_tiles):
                nv = min(P, rpc - t * P)
                bnz = wpool.tile([P, P], f32, tag="bnz")
                nc.vector.tensor_scalar(
                    out=bnz[:], in0=zbuf[:, t, :],
                    scalar1=gp[:], scalar2=sh[:],
                    op0=mybir.AluOpType.mult, op1=mybir.AluOpType.add)
                pT3 = psum.tile([P, P], f32, tag="pT3")
                nc.tensor.transpose(out=pT3[:], in_=bnz[:], identity=ident[:])
                yT = wpool.tile([P, P], f32, tag="yT")
                nc.scalar.copy(out=yT[:], in_=pT3[:])
                ysq = wpool.tile([P, P], f32, tag="ysq")
                n2 = wpool.tile([P, 1], f32, tag="n2")
                nc.scalar.activation(
                    out=ysq[:], in_=yT[:],
                    func=mybir.ActivationFunctionType.Square,
                    accum_out=n2[:])
                nrm = wpool.tile([P, 1], f32, tag="nrm")
                nc.scalar.sqrt(out=nrm[:], in_=n2[:])
                nc.vector.tensor_scalar_add(nrm[:], nrm[:], NORM_EPS)
                rn = wpool.tile([P, 1], f32, tag="rn")
                nc.vector.reciprocal(out=rn[:], in_=nrm[:])
                y = wpool.tile([P, P], f32, tag="y")
                nc.vector.tensor_scalar_mul(y[:], yT[:], rn[:])
                nc.sync.dma_start(out=out[t * P:t * P + nv, :],
                                  in_=y[0:nv, :])

    nc.compile()
    return nc


def _get_program(layout):
    key = (layout["S"], layout["instrs"], layout["vmax"], layout["rpc"])
    if key not in _prog_cache:
        _prog_cache[key] = build_program(layout)
    return _prog_cache[key]


# --------------------------------------------------------------------------
# Entry point
# --------------------------------------------------------------------------

def kernel(features, self_idx, neigh_idx, W, b, gamma, beta):
    features = np.ascontiguousarray(np.asarray(features, dtype=np.float32))
    self_idx = np.asarray(self_idx).astype(np.int64)
    neigh_idx = np.asarray(neigh_idx).astype(np.int64)
    W = np.asarray(W, dtype=np.float32)
    n, s = neigh_idx.shape
    table_rows, d = features.shape
    rpc = n // N_CORES

    # bf16 table with zero separators: [z C0 z C1 z ... C6 z]
    btab_rows = 1 + K_CHUNK * (CH + 1)
    btab = np.zeros((btab_rows, d), dtype=ml_dtypes.bfloat16)
    fb = features.astype(ml_dtypes.bfloat16)
    for c in range(K_CHUNK):
        r0 = c * CH
        r1 = min(table_rows, (c + 1) * CH)
        if r1 > r0:
            btab[1 + c * (CH + 1):1 + c * (CH + 1) + (r1 - r0)] = fb[r0:r1]

    w1t = np.ascontiguousarray(W[:, :d].T)
    w2ts = np.ascontiguousarray((W[:, d:] / float(s)).T)
    bvec = np.asarray(b, dtype=np.float32).reshape(d, 1).copy()
    gvec = np.asarray(gamma, dtype=np.float32).reshape(d, 1).copy()
    betav = np.asarray(beta, dtype=np.float32).reshape(d, 1).copy()

    layout, idx_slabs, perms = _build_layout(self_idx, neigh_idx)
    nc = _get_program(layout)

    in_maps = []
    for c in range(N_CORES):
        in_maps.append({
            "btab": btab,
            "idxslab": idx_slabs[c],
            "w1t": w1t,
            "w2ts": w2ts,
            "bvec": bvec,
            "gvec": gvec,
            "betav": betav,
        })

    global _last_in_maps
    _last_in_maps = in_maps
    res = run_bass_kernel_spmd(nc, in_maps, core_ids=list(range(N_CORES)))
    outp = np.empty((n, d), dtype=np.float32)
    for c in range(N_CORES):
        oc = res.results[c]["out"]
        outp[c * rpc:(c + 1) * rpc] = oc[perms[c]]
    return outp


_last_in_maps = None


# revision 19
# speedup vs baseline: 1.4039x; 1.4039x over previous
"""GraphSAGE layer on 8 Trainium2 NeuronCores (Bass/Tile).

Strategy: data-parallel over the 50000 target nodes (6250 rows/core),
feature table replicated per core as a bf16 copy with zero-row separators
between 7 chunks of <=28572 rows (so chunk-local row ids fit int16).
Neighbor + self rows are gathered with batched SWDGE dma_gather (one
instruction per (group, chunk, slot-batch), ~0.34ns/descriptor) instead of
per-128-row INDIRECT1D (~1.1us each). Per 4-tile group, each chunk owns a
quota of gather slots; unused positions carry idx=-1, which the ucode
resolves to (chunk base - 1 row) = a zero separator row, so dummy slots
contribute exact zeros to the slot-fold tree. Targets are greedily
grouped to minimize per-group chunk quotas. Per tile: bf16 in-place fold
tree over slots -> total sum; small tree over the 7 self slots -> self
row; agg = total - self. Then (as before) PE transposes + matmuls
(out.T = W1 @ self.T + (W2/25) @ agg.T), ReLU+bias with fused BN-stat
accumulation, AllReduce of (sum, sumsq), BN apply + row L2-normalize,
per-shard output written back and un-permuted on host.
"""
from contextlib import ExitStack

import numpy as np
import ml_dtypes

import concourse.bacc as bacc
import concourse.bass as bass
import concourse.tile as tile
from concourse import mybir
from concourse.bass_utils import run_bass_kernel_spmd
from concourse.masks import make_identity

BN_EPS = 1e-5
NORM_EPS = 1e-6

N_CORES = 8
N_TOTAL = 50000
TABLE_ROWS = 200000
D = 128
S_NEIGH = 25
P = 128
G = 4                    # tiles per group
K_CHUNK = 7
CH = 28572               # chunk rows (< 32768 for int16)
MAX_SLOT_BATCH = 14      # slots per gather instruction (num_idxs <= 7168)
S_CAP = 60               # max slots per group (SBUF: gbuf = S*G*256B/part)

_prog_cache = {}


# --------------------------------------------------------------------------
# Host-side layout construction
# --------------------------------------------------------------------------

def _greedy_group_joint(counts_all, caps):
    """Jointly assign each core's targets to groups against SHARED quota
    vectors, minimizing sum_c max(count_c) per group (max taken over the
    union of all cores' members of that group).

    counts_all: [n_cores, n, K]. caps: per-group per-core capacity.
    Returns assign[n_cores, n], pos[n_cores, n], gmax[ngroups, K].
    """
    n_cores, n, _ = counts_all.shape
    ngroups = len(caps)
    caps = np.asarray(caps, dtype=np.int64)
    gmax = np.zeros((ngroups, K_CHUNK), dtype=np.int64)
    fill = np.zeros((n_cores, ngroups), dtype=np.int64)
    assign = np.empty((n_cores, n), dtype=np.int64)
    pos = np.empty((n_cores, n), dtype=np.int64)
    slot_budget = S_CAP - K_CHUNK     # neighbor slots only (self = +7)
    big = np.iinfo(np.int64).max
    # interleave cores, targets in descending max-count order per core
    orders = [np.argsort(-counts_all[k].max(axis=1), kind="stable")
              for k in range(n_cores)]
    for j in range(n):
        for k in range(n_cores):
            i = orders[k][j]
            newmax = np.maximum(gmax, counts_all[k][i])
            delta = newmax.sum(axis=1) - gmax.sum(axis=1)
            bad = ((fill[k] >= caps)
                   | (newmax.sum(axis=1) > slot_budget))
            if bad.all():
                delta = np.where(fill[k] < caps, delta, big)
            else:
                delta = np.where(bad, big, delta)
            g = int(np.argmin(delta))
            assign[k][i] = g
            pos[k][i] = fill[k][g]
            fill[k][g] += 1
            gmax[g] = np.maximum(gmax[g], counts_all[k][i])
    return assign, pos, gmax


def _build_layout(self_idx, neigh_idx):
    """Compute the shared (across cores) slot layout + per-core idx slabs.

    Returns (layout, idx_slabs, perms):
      layout: dict with n_groups, quotas n[g][c], slot starts a[g][c], S[g],
        per-instruction list per group: (dst_slot0, nslots, col0, npos,
        nvalid), total idx columns.
      idx_slabs: [n_cores] arrays [128, totcols] int16.
      perms: [n_cores] arrays row_of_target[i] (row index in the core's
        out tensor for local target i).
    """
    n = self_idx.shape[0]
    rpc = n // N_CORES
    n_tiles = (rpc + P - 1) // P
    n_groups = (n_tiles + G - 1) // G
    caps = [min(G * P, max(0, rpc - g * G * P)) for g in range(n_groups)]

    nb_all = np.empty((N_CORES, rpc, S_NEIGH), dtype=np.int64)
    sf_all = np.empty((N_CORES, rpc), dtype=np.int64)
    counts_all = np.empty((N_CORES, rpc, K_CHUNK), dtype=np.int64)
    for k in range(N_CORES):
        sl = slice(k * rpc, (k + 1) * rpc)
        nb_all[k] = np.asarray(neigh_idx[sl], dtype=np.int64)
        sf_all[k] = np.asarray(self_idx[sl], dtype=np.int64)
        ch = nb_all[k] // CH
        counts_all[k] = (
            ch[:, :, None] == np.arange(K_CHUNK)[None, None, :]).sum(1)
    assign_all, pos_all, nq = _greedy_group_joint(counts_all, caps)
    per_core = [(nb_all[k], sf_all[k], assign_all[k], pos_all[k])
                for k in range(N_CORES)]

    # slot layout per group: chunk c -> [a_c, a_c + n_c) neighbors,
    # a_c + n_c = self slot; chunk 6 gets one extra always-zero pad slot
    # (pairs with the 7th self slot in the self fold tree).
    a = np.zeros((n_groups, K_CHUNK), dtype=np.int64)
    S = np.zeros(n_groups, dtype=np.int64)
    for g in range(n_groups):
        acc = 0
        for c in range(K_CHUNK):
            a[g, c] = acc
            acc += nq[g, c] + 1 + (1 if c == K_CHUNK - 1 else 0)
        S[g] = acc

    # per-group instruction list (split big chunks into slot batches)
    instrs = []           # per group: list of (c, slot0, nslots, col0, npos)
    gcol0 = []            # first column of each group's slab region
    col = 0
    for g in range(n_groups):
        gi = []
        gcol0.append(col)
        for c in range(K_CHUNK):
            total_slots = nq[g, c] + 1 + (1 if c == K_CHUNK - 1 else 0)
            s0 = a[g, c]
            while total_slots > 0:
                k = min(MAX_SLOT_BATCH, total_slots)
                npos = k * G * P
                gi.append((c, int(s0), int(k), int(col), int(npos)))
                col += npos // 16
                s0 += k
                total_slots -= k
        instrs.append(gi)
    totcols = col

    # per-core slabs + valid counts
    slabs = np.full((N_CORES, 16, totcols), -1, dtype=np.int16)
    valid = np.zeros((N_CORES, sum(len(gi) for gi in instrs)), dtype=np.int64)
    perms = []
    ii_of = {}
    ii = 0
    for g in range(n_groups):
        for j, ins in enumerate(instrs[g]):
            ii_of[(g, j)] = ii
            ii += 1

    for k in range(N_CORES):
        nb, sf, assign, pos = per_core[k]
        row_of = np.empty(rpc, dtype=np.int64)
        # position array per group: A[s, t, p]
        for g in range(n_groups):
            m = np.where(assign == g)[0]
            A = np.full((int(S[g]), G, P), -1, dtype=np.int64)
            if len(m) > 0:
                t_of = pos[m] // P
                p_of = pos[m] % P
                row_of[m] = (g * G + t_of) * P + p_of
                srt = np.sort(nb[m], axis=1)
                chs = srt // CH
                loc = srt - chs * CH
                # rank within chunk: count of earlier entries in same chunk
                rank = (np.arange(S_NEIGH)[None, :]
                        - (srt[:, None, :]
                           < (chs * CH)[:, :, None]).sum(-1))
                slot = a[g][chs] + rank
                A[slot.reshape(-1),
                  np.repeat(t_of, S_NEIGH),
                  np.repeat(p_of, S_NEIGH)] = loc.reshape(-1)
                sc = sf[m] // CH
                A[(a[g] + nq[g])[sc], t_of, p_of] = sf[m] - sc * CH
            for j, (c, s0, nsl, col0, npos) in enumerate(instrs[g]):
                blk = A[s0:s0 + nsl].reshape(-1).copy()
                # all positions must be valid for multi-packet gathers:
                # dummies read the trailing zero-separator row (idx CH)
                blk[blk < 0] = CH
                valid[k, ii_of[(g, j)]] = npos
                slabs[k, :, col0:col0 + npos // 16] = (
                    blk.reshape(-1, 16).T.astype(np.int16))
        perms.append(row_of)

    vmax = valid.max(axis=0)

    layout = {
        "n_groups": n_groups,
        "n_tiles": n_tiles,
        "rpc": rpc,
        "S": tuple(int(x) for x in S),
        "instrs": tuple(tuple(gi) for gi in instrs),
        "gcol0": tuple(gcol0),
        "totcols": totcols,
        "vmax": tuple(int(x) for x in vmax),
        "ii_of": ii_of,
        "a": tuple(tuple(int(x) for x in a[g]) for g in range(n_groups)),
        "nq": tuple(tuple(int(x) for x in nq[g]) for g in range(n_groups)),
    }
    idx_slabs = [np.tile(slabs[k], (8, 1)) for k in range(N_CORES)]
    return layout, idx_slabs, perms


# --------------------------------------------------------------------------
# Device program
# --------------------------------------------------------------------------

def build_program(layout):
    n_groups = layout["n_groups"]
    n_tiles = layout["n_tiles"]
    rpc = layout["rpc"]
    S = layout["S"]
    instrs = layout["instrs"]
    gcol0 = layout["gcol0"]
    totcols = layout["totcols"]
    vmax = layout["vmax"]
    ii_of = layout["ii_of"]
    maxS = max(S)
    btab_rows = 1 + K_CHUNK * (CH + 1)

    nc = bacc.Bacc("TRN2", target_bir_lowering=False, num_devices=N_CORES,
                   num_swdge_queues=4)
    f32 = mybir.dt.float32
    bf16 = mybir.dt.bfloat16
    btab = nc.dram_tensor("btab", [btab_rows, D], bf16, kind="ExternalInput")
    idxslab = nc.dram_tensor("idxslab", [P, totcols], mybir.dt.int16,
                             kind="ExternalInput")
    w1t = nc.dram_tensor("w1t", [D, D], f32, kind="ExternalInput")
    w2ts = nc.dram_tensor("w2ts", [D, D], f32, kind="ExternalInput")
    bvec = nc.dram_tensor("bvec", [D, 1], f32, kind="ExternalInput")
    gvec = nc.dram_tensor("gvec", [D, 1], f32, kind="ExternalInput")
    betav = nc.dram_tensor("betav", [D, 1], f32, kind="ExternalInput")
    out = nc.dram_tensor("out", [rpc, D], f32, kind="ExternalOutput")

    ar_in = nc.dram_tensor("ar_in", [D, 2], f32)
    ar_out = nc.dram_tensor("ar_out", [D, 2], f32, addr_space="Shared")

    with tile.TileContext(nc) as tc:
        with ExitStack() as ctx:
            singles = ctx.enter_context(tc.tile_pool(name="singles", bufs=1))
            gpool = ctx.enter_context(tc.tile_pool(name="gpool", bufs=2))
            ipool = ctx.enter_context(tc.tile_pool(name="ipool", bufs=2))
            wpool = ctx.enter_context(tc.tile_pool(name="wpool", bufs=3))
            psum = ctx.enter_context(tc.tile_pool(name="psum", bufs=2,
                                                  space="PSUM"))
            psum2 = ctx.enter_context(tc.tile_pool(name="psum2", bufs=2,
                                                   space="PSUM"))

            w1t_sb = singles.tile([D, D], f32)
            nc.sync.dma_start(out=w1t_sb[:], in_=w1t[:])
            w2ts_sb = singles.tile([D, D], f32)
            nc.sync.dma_start(out=w2ts_sb[:], in_=w2ts[:])
            b_sb = singles.tile([D, 1], f32)
            nc.sync.dma_start(out=b_sb[:], in_=bvec[:])
            g_sb = singles.tile([D, 1], f32)
            nc.sync.dma_start(out=g_sb[:], in_=gvec[:])
            beta_sb = singles.tile([D, 1], f32)
            nc.sync.dma_start(out=beta_sb[:], in_=betav[:])
            ident = singles.tile([P, P], f32)
            make_identity(nc, ident[:])

            zbuf = singles.tile([P, n_tiles, P], f32)
            sums = singles.tile([P, n_tiles], f32)
            sumsq = singles.tile([P, n_tiles], f32)

            maxcols = max(
                sum(npos // 16 for (_, _, _, _, npos) in instrs[g])
                for g in range(n_groups))

            # ---------------- Phase A ------------------------------------
            for g in range(n_groups):
                Sg = S[g]
                cols_g = sum(npos // 16 for (_, _, _, _, npos) in instrs[g])
                idxt = ipool.tile([P, maxcols], mybir.dt.int16, tag="idxt")
                nc.sync.dma_start(
                    out=idxt[:, 0:cols_g],
                    in_=idxslab[:, gcol0[g]:gcol0[g] + cols_g])

                gbuf = gpool.tile([P, maxS * G, D], bf16, tag="gbuf")
                for j, (c, s0, nsl, col0, npos) in enumerate(instrs[g]):
                    base_c = 1 + c * (CH + 1)
                    lc0 = col0 - gcol0[g]
                    nc.gpsimd.dma_gather(
                        out_ap=gbuf[:, s0 * G:(s0 + nsl) * G, :],
                        in_ap=btab[base_c:base_c + CH + 1, :],
                        idxs_ap=idxt[:, lc0:lc0 + npos // 16],
                        num_idxs=npos,
                        num_idxs_reg=vmax[ii_of[(g, j)]],
                        elem_size=D,
                        single_packet=False,
                        queue_num=j % 4)

                gview = gbuf[:].rearrange("p (s g) d -> p s g d", g=G)
                # self slot per chunk = a_c + n_c; chunk 6 has an
                # always-zero pad slot right after its self slot
                self_slots = [layout["a"][g][c] + layout["nq"][g][c]
                              for c in range(K_CHUNK)]
                pad_slot = self_slots[K_CHUNK - 1] + 1

                for t in range(G):
                    tg = g * G + t
                    if tg >= n_tiles:
                        continue
                    nv = min(P, rpc - tg * P)

                    # self sum over the 7 self slots + zero pad slot
                    st = wpool.tile([P, 4, D], f32, tag="st")
                    nc.vector.tensor_add(
                        st[:, 0, :],
                        gview[:, self_slots[0], t, :],
                        gview[:, self_slots[1], t, :])
                    nc.vector.tensor_add(
                        st[:, 1, :],
                        gview[:, self_slots[2], t, :],
                        gview[:, self_slots[3], t, :])
                    nc.vector.tensor_add(
                        st[:, 2, :],
                        gview[:, self_slots[4], t, :],
                        gview[:, self_slots[5], t, :])
                    nc.vector.tensor_add(
                        st[:, 3, :],
                        gview[:, self_slots[6], t, :],
                        gview[:, pad_slot, t, :])
                    selfsum = wpool.tile([P, D], f32, tag="selfsum")
                    nc.vector.tensor_add(st[:, 0, :], st[:, 0, :],
                                         st[:, 1, :])
                    nc.vector.tensor_add(st[:, 2, :], st[:, 2, :],
                                         st[:, 3, :])
                    nc.vector.tensor_add(selfsum[:], st[:, 0, :],
                                         st[:, 2, :])

                    # in-place bf16 fold tree over all Sg slots
                    cur = Sg
                    while cur > 2:
                        if cur % 2 == 1:
                            nc.vector.tensor_add(
                                gview[:, 0, t, :],
                                gview[:, 0, t, :],
                                gview[:, cur - 1, t, :])
                            cur -= 1
                            if cur == 2:
                                break
                        h = cur // 2
                        nc.vector.tensor_add(
                            gview[:, 0:h, t, :],
                            gview[:, 0:h, t, :],
                            gview[:, h:cur, t, :])
                        cur = h
                    sall = wpool.tile([P, D], f32, tag="sall")
                    nc.vector.tensor_add(sall[:], gview[:, 0, t, :],
                                         gview[:, 1, t, :])
                    agg = wpool.tile([P, D], f32, tag="agg")
                    nc.vector.tensor_sub(agg[:], sall[:], selfsum[:])

                    # transposes via PE
                    pT = psum.tile([P, P], f32, tag="pT")
                    nc.tensor.transpose(out=pT[:], in_=selfsum[:],
                                        identity=ident[:])
                    sT = wpool.tile([P, P], f32, tag="sT")
                    nc.scalar.copy(out=sT[:], in_=pT[:])
                    pT2 = psum.tile([P, P], f32, tag="pT2")
                    nc.tensor.transpose(out=pT2[:], in_=agg[:],
                                        identity=ident[:])
                    aT = wpool.tile([P, P], f32, tag="aT")
                    nc.scalar.copy(out=aT[:], in_=pT2[:])

                    mm = psum2.tile([P, P], f32, tag="mm")
                    nc.tensor.matmul(mm[:], w1t_sb[:], sT[:],
                                     start=True, stop=False)
                    nc.tensor.matmul(mm[:], w2ts_sb[:], aT[:],
                                     start=False, stop=True)

                    if nv == P:
                        nc.scalar.activation(
                            out=zbuf[:, tg, :], in_=mm[:],
                            func=mybir.ActivationFunctionType.Relu,
                            bias=b_sb[:], scale=1.0,
                            accum_out=sums[:, tg:tg + 1])
                        dump = wpool.tile([P, P], f32, tag="dump")
                        nc.scalar.activation(
                            out=dump[:], in_=zbuf[:, tg, :],
                            func=mybir.ActivationFunctionType.Square,
                            accum_out=sumsq[:, tg:tg + 1])
                    else:
                        nc.scalar.activation(
                            out=zbuf[:, tg, 0:nv], in_=mm[:, 0:nv],
                            func=mybir.ActivationFunctionType.Relu,
                            bias=b_sb[:], scale=1.0,
                            accum_out=sums[:, tg:tg + 1])
                        dump = wpool.tile([P, P], f32, tag="dump")
                        nc.scalar.activation(
                            out=dump[:, 0:nv], in_=zbuf[:, tg, 0:nv],
                            func=mybir.ActivationFunctionType.Square,
                            accum_out=sumsq[:, tg:tg + 1])

            # ---------------- Phase B: global BN stats -------------------
            gstat = singles.tile([P, 2], f32)
            nc.vector.tensor_reduce(out=gstat[:, 0:1], in_=sums[:],
                                    axis=mybir.AxisListType.X,
                                    op=mybir.AluOpType.add)
            nc.vector.tensor_reduce(out=gstat[:, 1:2], in_=sumsq[:],
                                    axis=mybir.AxisListType.X,
                                    op=mybir.AluOpType.add)
            nc.sync.dma_start(out=ar_in[:], in_=gstat[:])
            nc.gpsimd.collective_compute(
                "AllReduce", mybir.AluOpType.add,
                ins=[ar_in[:]],
                outs=[ar_out[:]],
                replica_groups=[list(range(N_CORES))],
            )
            gg = singles.tile([P, 2], f32)
            nc.sync.dma_start(out=gg[:], in_=ar_out[:])

            inv_n = 1.0 / float(N_TOTAL)
            mu = singles.tile([P, 1], f32)
            nc.vector.tensor_scalar_mul(mu[:], gg[:, 0:1], inv_n)
            ex2 = singles.tile([P, 1], f32)
            nc.vector.tensor_scalar_mul(ex2[:], gg[:, 1:2], inv_n)
            var = singles.tile([P, 1], f32)
            nc.vector.tensor_mul(var[:], mu[:], mu[:])
            nc.vector.tensor_sub(var[:], ex2[:], var[:])
            nc.vector.tensor_scalar_add(var[:], var[:], BN_EPS)
            std = singles.tile([P, 1], f32)
            nc.scalar.sqrt(out=std[:], in_=var[:])
            rstd = singles.tile([P, 1], f32)
            nc.vector.reciprocal(out=rstd[:], in_=std[:])
            gp = singles.tile([P, 1], f32)
            nc.vector.tensor_mul(gp[:], g_sb[:], rstd[:])
            sh = singles.tile([P, 1], f32)
            nc.vector.tensor_mul(sh[:], mu[:], gp[:])
            nc.vector.tensor_sub(sh[:], beta_sb[:], sh[:])

            # ---------------- Phase C: BN apply + L2 normalize -----------
            for t in range(n_tiles):
                nv = min(P, rpc - t * P)
                bnz = wpool.tile([P, P], f32, tag="bnz")
                nc.vector.tensor_scalar(
                    out=bnz[:], in0=zbuf[:, t, :],
                    scalar1=gp[:], scalar2=sh[:],
                    op0=mybir.AluOpType.mult, op1=mybir.AluOpType.add)
                pT3 = psum.tile([P, P], f32, tag="pT3")
                nc.tensor.transpose(out=pT3[:], in_=bnz[:], identity=ident[:])
                yT = wpool.tile([P, P], f32, tag="yT")
                nc.scalar.copy(out=yT[:], in_=pT3[:])
                ysq = wpool.tile([P, P], f32, tag="ysq")
                n2 = wpool.tile([P, 1], f32, tag="n2")
                nc.scalar.activation(
                    out=ysq[:], in_=yT[:],
                    func=mybir.ActivationFunctionType.Square,
                    accum_out=n2[:])
                nrm = wpool.tile([P, 1], f32, tag="nrm")
                nc.scalar.sqrt(out=nrm[:], in_=n2[:])
                nc.vector.tensor_scalar_add(nrm[:], nrm[:], NORM_EPS)
                rn = wpool.tile([P, 1], f32, tag="rn")
                nc.vector.reciprocal(out=rn[:], in_=nrm[:])
                y = wpool.tile([P, P], f32, tag="y")
                nc.vector.tensor_scalar_mul(y[:], yT[:], rn[:])
                nc.sync.dma_start(out=out[t * P:t * P + nv, :],
                                  in_=y[0:nv, :])

    nc.compile()
    return nc


def _get_program(layout):
    key = (layout["S"], layout["instrs"], layout["vmax"], layout["rpc"])
    if key not in _prog_cache:
        _prog_cache[key] = build_program(layout)
    return _prog_cache[key]


# --------------------------------------------------------------------------
# Entry point
# --------------------------------------------------------------------------

def kernel(features, self_idx, neigh_idx, W, b, gamma, beta):
    features = np.ascontiguousarray(np.asarray(features, dtype=np.float32))
    self_idx = np.asarray(self_idx).astype(np.int64)
    neigh_idx = np.asarray(neigh_idx).astype(np.int64)
    W = np.asarray(W, dtype=np.float32)
    n, s = neigh_idx.shape
    table_rows, d = features.shape
    rpc = n // N_CORES

    # bf16 table with zero separators: [z C0 z C1 z ... C6 z]
    btab_rows = 1 + K_CHUNK * (CH + 1)
    btab = np.zeros((btab_rows, d), dtype=ml_dtypes.bfloat16)
    fb = features.astype(ml_dtypes.bfloat16)
    for c in range(K_CHUNK):
        r0 = c * CH
        r1 = min(table_rows, (c + 1) * CH)
        if r1 > r0:
            btab[1 + c * (CH + 1):1 + c * (CH + 1) + (r1 - r0)] = fb[r0:r1]

    w1t = np.ascontiguousarray(W[:, :d].T)
    w2ts = np.ascontiguousarray((W[:, d:] / float(s)).T)
    bvec = np.asarray(b, dtype=np.float32).reshape(d, 1).copy()
    gvec = np.asarray(gamma, dtype=np.float32).reshape(d, 1).copy()
    betav = np.asarray(beta, dtype=np.float32).reshape(d, 1).copy()

    layout, idx_slabs, perms = _build_layout(self_idx, neigh_idx)
    nc = _get_program(layout)

    in_maps = []
    for c in range(N_CORES):
        in_maps.append({
            "btab": btab,
            "idxslab": idx_slabs[c],
            "w1t": w1t,
            "w2ts": w2ts,
            "bvec": bvec,
            "gvec": gvec,
            "betav": betav,
        })

    global _last_in_maps
    _last_in_maps = in_maps
    res = run_bass_kernel_spmd(nc, in_maps, core_ids=list(range(N_CORES)))
    outp = np.empty((n, d), dtype=np.float32)
    for c in range(N_CORES):
        oc = res.results[c]["out"]
        outp[c * rpc:(c + 1) * rpc] = oc[perms[c]]
    return outp


_last_in_maps = None


# revision 20
# speedup vs baseline: 1.5650x; 1.1148x over previous
"""GraphSAGE layer on 8 Trainium2 NeuronCores (Bass/Tile).

Strategy: data-parallel over the 50000 target nodes (6250 rows/core),
feature table replicated per core as a bf16 copy with zero-row separators
between 7 chunks of <=28572 rows (so chunk-local row ids fit int16).
Neighbor + self rows are gathered with batched SWDGE dma_gather (one
instruction per (group, chunk, slot-batch), ~0.34ns/descriptor) instead of
per-128-row INDIRECT1D (~1.1us each). Per 4-tile group, each chunk owns a
quota of gather slots; unused positions carry idx=-1, which the ucode
resolves to (chunk base - 1 row) = a zero separator row, so dummy slots
contribute exact zeros to the slot-fold tree. Targets are greedily
grouped to minimize per-group chunk quotas. Per tile: bf16 in-place fold
tree over slots -> total sum; small tree over the 7 self slots -> self
row; agg = total - self. Then (as before) PE transposes + matmuls
(out.T = W1 @ self.T + (W2/25) @ agg.T), ReLU+bias with fused BN-stat
accumulation, AllReduce of (sum, sumsq), BN apply + row L2-normalize,
per-shard output written back and un-permuted on host.
"""
from contextlib import ExitStack

import numpy as np
import ml_dtypes

import concourse.bacc as bacc
import concourse.bass as bass
import concourse.tile as tile
from concourse import mybir
from concourse.bass_utils import run_bass_kernel_spmd
from concourse.masks import make_identity

BN_EPS = 1e-5
NORM_EPS = 1e-6

N_CORES = 8
N_TOTAL = 50000
TABLE_ROWS = 200000
D = 128
S_NEIGH = 25
P = 128
G = 2                    # tiles per group
K_CHUNK = 7
CH = 28572               # chunk rows (< 32768 for int16)
MAX_SLOT_BATCH = 14      # slots per gather instruction (num_idxs <= 7168)
S_CAP = 48               # max slots per group (SBUF: gbuf = S*G*256B/part)

_prog_cache = {}


# --------------------------------------------------------------------------
# Host-side layout construction
# --------------------------------------------------------------------------

def _greedy_group_joint(counts_all, caps):
    """Jointly assign each core's targets to groups against SHARED quota
    vectors, minimizing sum_c max(count_c) per group (max taken over the
    union of all cores' members of that group).

    counts_all: [n_cores, n, K]. caps: per-group per-core capacity.
    Returns assign[n_cores, n], pos[n_cores, n], gmax[ngroups, K].
    """
    n_cores, n, _ = counts_all.shape
    ngroups = len(caps)
    caps = np.asarray(caps, dtype=np.int64)
    gmax = np.zeros((ngroups, K_CHUNK), dtype=np.int64)
    fill = np.zeros((n_cores, ngroups), dtype=np.int64)
    assign = np.empty((n_cores, n), dtype=np.int64)
    pos = np.empty((n_cores, n), dtype=np.int64)
    slot_budget = S_CAP - K_CHUNK     # neighbor slots only (self = +7)
    big = np.iinfo(np.int64).max
    # interleave cores, targets in descending max-count order per core
    orders = [np.argsort(-counts_all[k].max(axis=1), kind="stable")
              for k in range(n_cores)]
    for j in range(n):
        for k in range(n_cores):
            i = orders[k][j]
            newmax = np.maximum(gmax, counts_all[k][i])
            delta = newmax.sum(axis=1) - gmax.sum(axis=1)
            bad = ((fill[k] >= caps)
                   | (newmax.sum(axis=1) > slot_budget))
            if bad.all():
                delta = np.where(fill[k] < caps, delta, big)
            else:
                delta = np.where(bad, big, delta)
            g = int(np.argmin(delta))
            assign[k][i] = g
            pos[k][i] = fill[k][g]
            fill[k][g] += 1
            gmax[g] = np.maximum(gmax[g], counts_all[k][i])
    return assign, pos, gmax


def _build_layout(self_idx, neigh_idx):
    """Compute the shared (across cores) slot layout + per-core idx slabs.

    Returns (layout, idx_slabs, perms):
      layout: dict with n_groups, quotas n[g][c], slot starts a[g][c], S[g],
        per-instruction list per group: (dst_slot0, nslots, col0, npos,
        nvalid), total idx columns.
      idx_slabs: [n_cores] arrays [128, totcols] int16.
      perms: [n_cores] arrays row_of_target[i] (row index in the core's
        out tensor for local target i).
    """
    n = self_idx.shape[0]
    rpc = n // N_CORES
    n_tiles = (rpc + P - 1) // P
    n_groups = (n_tiles + G - 1) // G
    caps = [min(G * P, max(0, rpc - g * G * P)) for g in range(n_groups)]

    nb_all = np.empty((N_CORES, rpc, S_NEIGH), dtype=np.int64)
    sf_all = np.empty((N_CORES, rpc), dtype=np.int64)
    counts_all = np.empty((N_CORES, rpc, K_CHUNK), dtype=np.int64)
    for k in range(N_CORES):
        sl = slice(k * rpc, (k + 1) * rpc)
        nb_all[k] = np.asarray(neigh_idx[sl], dtype=np.int64)
        sf_all[k] = np.asarray(self_idx[sl], dtype=np.int64)
        ch = nb_all[k] // CH
        counts_all[k] = (
            ch[:, :, None] == np.arange(K_CHUNK)[None, None, :]).sum(1)
    assign_all, pos_all, nq = _greedy_group_joint(counts_all, caps)
    per_core = [(nb_all[k], sf_all[k], assign_all[k], pos_all[k])
                for k in range(N_CORES)]

    # slot layout per group: chunk c -> [a_c, a_c + n_c) neighbors,
    # a_c + n_c = self slot; chunk 6 gets one extra always-zero pad slot
    # (pairs with the 7th self slot in the self fold tree).
    a = np.zeros((n_groups, K_CHUNK), dtype=np.int64)
    S = np.zeros(n_groups, dtype=np.int64)
    for g in range(n_groups):
        acc = 0
        for c in range(K_CHUNK):
            a[g, c] = acc
            acc += nq[g, c] + 1 + (1 if c == K_CHUNK - 1 else 0)
        S[g] = acc

    # per-group instruction list (split big chunks into slot batches)
    instrs = []           # per group: list of (c, slot0, nslots, col0, npos)
    gcol0 = []            # first column of each group's slab region
    col = 0
    for g in range(n_groups):
        gi = []
        gcol0.append(col)
        for c in range(K_CHUNK):
            total_slots = nq[g, c] + 1 + (1 if c == K_CHUNK - 1 else 0)
            s0 = a[g, c]
            while total_slots > 0:
                k = min(MAX_SLOT_BATCH, total_slots)
                npos = k * G * P
                gi.append((c, int(s0), int(k), int(col), int(npos)))
                col += npos // 16
                s0 += k
                total_slots -= k
        instrs.append(gi)
    totcols = col

    # per-core slabs + valid counts
    slabs = np.full((N_CORES, 16, totcols), -1, dtype=np.int16)
    valid = np.zeros((N_CORES, sum(len(gi) for gi in instrs)), dtype=np.int64)
    perms = []
    ii_of = {}
    ii = 0
    for g in range(n_groups):
        for j, ins in enumerate(instrs[g]):
            ii_of[(g, j)] = ii
            ii += 1

    for k in range(N_CORES):
        nb, sf, assign, pos = per_core[k]
        row_of = np.empty(rpc, dtype=np.int64)
        # position array per group: A[s, t, p]
        for g in range(n_groups):
            m = np.where(assign == g)[0]
            A = np.full((int(S[g]), G, P), -1, dtype=np.int64)
            if len(m) > 0:
                t_of = pos[m] // P
                p_of = pos[m] % P
                row_of[m] = (g * G + t_of) * P + p_of
                srt = np.sort(nb[m], axis=1)
                chs = srt // CH
                loc = srt - chs * CH
                # rank within chunk: count of earlier entries in same chunk
                rank = (np.arange(S_NEIGH)[None, :]
                        - (srt[:, None, :]
                           < (chs * CH)[:, :, None]).sum(-1))
                slot = a[g][chs] + rank
                A[slot.reshape(-1),
                  np.repeat(t_of, S_NEIGH),
                  np.repeat(p_of, S_NEIGH)] = loc.reshape(-1)
                sc = sf[m] // CH
                A[(a[g] + nq[g])[sc], t_of, p_of] = sf[m] - sc * CH
            for j, (c, s0, nsl, col0, npos) in enumerate(instrs[g]):
                blk = A[s0:s0 + nsl].reshape(-1).copy()
                # all positions must be valid for multi-packet gathers:
                # dummies read the trailing zero-separator row (idx CH)
                blk[blk < 0] = CH
                valid[k, ii_of[(g, j)]] = npos
                slabs[k, :, col0:col0 + npos // 16] = (
                    blk.reshape(-1, 16).T.astype(np.int16))
        perms.append(row_of)

    vmax = valid.max(axis=0)

    layout = {
        "n_groups": n_groups,
        "n_tiles": n_tiles,
        "rpc": rpc,
        "S": tuple(int(x) for x in S),
        "instrs": tuple(tuple(gi) for gi in instrs),
        "gcol0": tuple(gcol0),
        "totcols": totcols,
        "vmax": tuple(int(x) for x in vmax),
        "ii_of": ii_of,
        "a": tuple(tuple(int(x) for x in a[g]) for g in range(n_groups)),
        "nq": tuple(tuple(int(x) for x in nq[g]) for g in range(n_groups)),
    }
    idx_slabs = [np.tile(slabs[k], (8, 1)) for k in range(N_CORES)]
    return layout, idx_slabs, perms


# --------------------------------------------------------------------------
# Device program
# --------------------------------------------------------------------------

def build_program(layout):
    n_groups = layout["n_groups"]
    n_tiles = layout["n_tiles"]
    rpc = layout["rpc"]
    S = layout["S"]
    instrs = layout["instrs"]
    gcol0 = layout["gcol0"]
    totcols = layout["totcols"]
    vmax = layout["vmax"]
    ii_of = layout["ii_of"]
    maxS = max(S)
    btab_rows = 1 + K_CHUNK * (CH + 1)

    nc = bacc.Bacc("TRN2", target_bir_lowering=False, num_devices=N_CORES,
                   num_swdge_queues=4)
    f32 = mybir.dt.float32
    bf16 = mybir.dt.bfloat16
    btab = nc.dram_tensor("btab", [btab_rows, D], bf16, kind="ExternalInput")
    idxslab = nc.dram_tensor("idxslab", [P, totcols], mybir.dt.int16,
                             kind="ExternalInput")
    w1t = nc.dram_tensor("w1t", [D, D], f32, kind="ExternalInput")
    w2ts = nc.dram_tensor("w2ts", [D, D], f32, kind="ExternalInput")
    bvec = nc.dram_tensor("bvec", [D, 1], f32, kind="ExternalInput")
    gvec = nc.dram_tensor("gvec", [D, 1], f32, kind="ExternalInput")
    betav = nc.dram_tensor("betav", [D, 1], f32, kind="ExternalInput")
    out = nc.dram_tensor("out", [rpc, D], f32, kind="ExternalOutput")

    ar_in = nc.dram_tensor("ar_in", [D, 2], f32)
    ar_out = nc.dram_tensor("ar_out", [D, 2], f32, addr_space="Shared")

    with tile.TileContext(nc) as tc:
        with ExitStack() as ctx:
            singles = ctx.enter_context(tc.tile_pool(name="singles", bufs=1))
            gpool = ctx.enter_context(tc.tile_pool(name="gpool", bufs=4))
            ipool = ctx.enter_context(tc.tile_pool(name="ipool", bufs=4))
            wpool = ctx.enter_context(tc.tile_pool(name="wpool", bufs=3))
            psum = ctx.enter_context(tc.tile_pool(name="psum", bufs=2,
                                                  space="PSUM"))
            psum2 = ctx.enter_context(tc.tile_pool(name="psum2", bufs=2,
                                                   space="PSUM"))

            w1t_sb = singles.tile([D, D], f32)
            nc.sync.dma_start(out=w1t_sb[:], in_=w1t[:])
            w2ts_sb = singles.tile([D, D], f32)
            nc.sync.dma_start(out=w2ts_sb[:], in_=w2ts[:])
            b_sb = singles.tile([D, 1], f32)
            nc.sync.dma_start(out=b_sb[:], in_=bvec[:])
            g_sb = singles.tile([D, 1], f32)
            nc.sync.dma_start(out=g_sb[:], in_=gvec[:])
            beta_sb = singles.tile([D, 1], f32)
            nc.sync.dma_start(out=beta_sb[:], in_=betav[:])
            ident = singles.tile([P, P], f32)
            make_identity(nc, ident[:])

            zbuf = singles.tile([P, n_tiles, P], f32)
            sums = singles.tile([P, n_tiles], f32)
            sumsq = singles.tile([P, n_tiles], f32)

            maxcols = max(
                sum(npos // 16 for (_, _, _, _, npos) in instrs[g])
                for g in range(n_groups))

            # ---------------- Phase A ------------------------------------
            for g in range(n_groups):
                Sg = S[g]
                cols_g = sum(npos // 16 for (_, _, _, _, npos) in instrs[g])
                idxt = ipool.tile([P, maxcols], mybir.dt.int16, tag="idxt")
                nc.sync.dma_start(
                    out=idxt[:, 0:cols_g],
                    in_=idxslab[:, gcol0[g]:gcol0[g] + cols_g])

                gbuf = gpool.tile([P, maxS * G, D], bf16, tag="gbuf")
                for j, (c, s0, nsl, col0, npos) in enumerate(instrs[g]):
                    base_c = 1 + c * (CH + 1)
                    lc0 = col0 - gcol0[g]
                    nc.gpsimd.dma_gather(
                        out_ap=gbuf[:, s0 * G:(s0 + nsl) * G, :],
                        in_ap=btab[base_c:base_c + CH + 1, :],
                        idxs_ap=idxt[:, lc0:lc0 + npos // 16],
                        num_idxs=npos,
                        num_idxs_reg=vmax[ii_of[(g, j)]],
                        elem_size=D,
                        single_packet=False,
                        queue_num=j % 4)

                gview = gbuf[:].rearrange("p (s g) d -> p s g d", g=G)
                # self slot per chunk = a_c + n_c; chunk 6 has an
                # always-zero pad slot right after its self slot
                self_slots = [layout["a"][g][c] + layout["nq"][g][c]
                              for c in range(K_CHUNK)]
                pad_slot = self_slots[K_CHUNK - 1] + 1

                for t in range(G):
                    tg = g * G + t
                    if tg >= n_tiles:
                        continue
                    nv = min(P, rpc - tg * P)

                    # self sum over the 7 self slots + zero pad slot
                    st = wpool.tile([P, 4, D], f32, tag="st")
                    nc.vector.tensor_add(
                        st[:, 0, :],
                        gview[:, self_slots[0], t, :],
                        gview[:, self_slots[1], t, :])
                    nc.vector.tensor_add(
                        st[:, 1, :],
                        gview[:, self_slots[2], t, :],
                        gview[:, self_slots[3], t, :])
                    nc.vector.tensor_add(
                        st[:, 2, :],
                        gview[:, self_slots[4], t, :],
                        gview[:, self_slots[5], t, :])
                    nc.vector.tensor_add(
                        st[:, 3, :],
                        gview[:, self_slots[6], t, :],
                        gview[:, pad_slot, t, :])
                    selfsum = wpool.tile([P, D], f32, tag="selfsum")
                    nc.vector.tensor_add(st[:, 0, :], st[:, 0, :],
                                         st[:, 1, :])
                    nc.vector.tensor_add(st[:, 2, :], st[:, 2, :],
                                         st[:, 3, :])
                    nc.vector.tensor_add(selfsum[:], st[:, 0, :],
                                         st[:, 2, :])

                    # in-place bf16 fold tree over all Sg slots
                    cur = Sg
                    while cur > 2:
                        if cur % 2 == 1:
                            nc.vector.tensor_add(
                                gview[:, 0, t, :],
                                gview[:, 0, t, :],
                                gview[:, cur - 1, t, :])
                            cur -= 1
                            if cur == 2:
                                break
                        h = cur // 2
                        nc.vector.tensor_add(
                            gview[:, 0:h, t, :],
                            gview[:, 0:h, t, :],
                            gview[:, h:cur, t, :])
                        cur = h
                    sall = wpool.tile([P, D], f32, tag="sall")
                    nc.vector.tensor_add(sall[:], gview[:, 0, t, :],
                                         gview[:, 1, t, :])
                    agg = wpool.tile([P, D], f32, tag="agg")
                    nc.vector.tensor_sub(agg[:], sall[:], selfsum[:])

                    # transposes via PE
                    pT = psum.tile([P, P], f32, tag="pT")
                    nc.tensor.transpose(out=pT[:], in_=selfsum[:],
                                        identity=ident[:])
                    sT = wpool.tile([P, P], f32, tag="sT")
                    nc.scalar.copy(out=sT[:], in_=pT[:])
                    pT2 = psum.tile([P, P], f32, tag="pT2")
                    nc.tensor.transpose(out=pT2[:], in_=agg[:],
                                        identity=ident[:])
                    aT = wpool.tile([P, P], f32, tag="aT")
                    nc.scalar.copy(out=aT[:], in_=pT2[:])

                    mm = psum2.tile([P, P], f32, tag="mm")
                    nc.tensor.matmul(mm[:], w1t_sb[:], sT[:],
                                     start=True, stop=False)
                    nc.tensor.matmul(mm[:], w2ts_sb[:], aT[:],
                                     start=False, stop=True)

                    if nv == P:
                        nc.scalar.activation(
                            out=zbuf[:, tg, :], in_=mm[:],
                            func=mybir.ActivationFunctionType.Relu,
                            bias=b_sb[:], scale=1.0,
                            accum_out=sums[:, tg:tg + 1])
                        dump = wpool.tile([P, P], f32, tag="dump")
                        nc.scalar.activation(
                            out=dump[:], in_=zbuf[:, tg, :],
                            func=mybir.ActivationFunctionType.Square,
                            accum_out=sumsq[:, tg:tg + 1])
                    else:
                        nc.scalar.activation(
                            out=zbuf[:, tg, 0:nv], in_=mm[:, 0:nv],
                            func=mybir.ActivationFunctionType.Relu,
                            bias=b_sb[:], scale=1.0,
                            accum_out=sums[:, tg:tg + 1])
                        dump = wpool.tile([P, P], f32, tag="dump")
                        nc.scalar.activation(
                            out=dump[:, 0:nv], in_=zbuf[:, tg, 0:nv],
                            func=mybir.ActivationFunctionType.Square,
                            accum_out=sumsq[:, tg:tg + 1])

            # ---------------- Phase B: global BN stats -------------------
            gstat = singles.tile([P, 2], f32)
            nc.vector.tensor_reduce(out=gstat[:, 0:1], in_=sums[:],
                                    axis=mybir.AxisListType.X,
                                    op=mybir.AluOpType.add)
            nc.vector.tensor_reduce(out=gstat[:, 1:2], in_=sumsq[:],
                                    axis=mybir.AxisListType.X,
                                    op=mybir.AluOpType.add)
            nc.sync.dma_start(out=ar_in[:], in_=gstat[:])
            nc.gpsimd.collective_compute(
                "AllReduce", mybir.AluOpType.add,
                ins=[ar_in[:]],
                outs=[ar_out[:]],
                replica_groups=[list(range(N_CORES))],
            )
            gg = singles.tile([P, 2], f32)
            nc.sync.dma_start(out=gg[:], in_=ar_out[:])

            inv_n = 1.0 / float(N_TOTAL)
            mu = singles.tile([P, 1], f32)
            nc.vector.tensor_scalar_mul(mu[:], gg[:, 0:1], inv_n)
            ex2 = singles.tile([P, 1], f32)
            nc.vector.tensor_scalar_mul(ex2[:], gg[:, 1:2], inv_n)
            var = singles.tile([P, 1], f32)
            nc.vector.tensor_mul(var[:], mu[:], mu[:])
            nc.vector.tensor_sub(var[:], ex2[:], var[:])
            nc.vector.tensor_scalar_add(var[:], var[:], BN_EPS)
            std = singles.tile([P, 1], f32)
            nc.scalar.sqrt(out=std[:], in_=var[:])
            rstd = singles.tile([P, 1], f32)
            nc.vector.reciprocal(out=rstd[:], in_=std[:])
            gp = singles.tile([P, 1], f32)
            nc.vector.tensor_mul(gp[:], g_sb[:], rstd[:])
            sh = singles.tile([P, 1], f32)
            nc.vector.tensor_mul(sh[:], mu[:], gp[:])
            nc.vector.tensor_sub(sh[:], beta_sb[:], sh[:])

            # ---------------- Phase C: BN apply + L2 normalize -----------
            for t in range(n_tiles):
                nv = min(P, rpc - t * P)
                bnz = wpool.tile([P, P], f32, tag="bnz")
                nc.vector.tensor_scalar(
                    out=bnz[:], in0=zbuf[:, t, :],
                    scalar1=gp[:], scalar2=sh[:],
                    op0=mybir.AluOpType.mult, op1=mybir.AluOpType.add)
                pT3 = psum.tile([P, P], f32, tag="pT3")
                nc.tensor.transpose(out=pT3[:], in_=bnz[:], identity=ident[:])
                yT = wpool.tile([P, P], f32, tag="yT")
                nc.scalar.copy(out=yT[:], in_=pT3[:])
                ysq = wpool.tile([P, P], f32, tag="ysq")
                n2 = wpool.tile([P, 1], f32, tag="n2")
                nc.scalar.activation(
                    out=ysq[:], in_=yT[:],
                    func=mybir.ActivationFunctionType.Square,
                    accum_out=n2[:])
                nrm = wpool.tile([P, 1], f32, tag="nrm")
                nc.scalar.sqrt(out=nrm[:], in_=n2[:])
                nc.vector.tensor_scalar_add(nrm[:], nrm[:], NORM_EPS)
                rn = wpool.tile([P, 1], f32, tag="rn")
                nc.vector.reciprocal(out=rn[:], in_=nrm[:])
                y = wpool.tile([P, P], f32, tag="y")
                nc.vector.tensor_scalar_mul(y[:], yT[:], rn[:])
                nc.sync.dma_start(out=out[t * P:t * P + nv, :],
                                  in_=y[0:nv, :])

    nc.compile()
    return nc


def _get_program(layout):
    key = (layout["S"], layout["instrs"], layout["vmax"], layout["rpc"])
    if key not in _prog_cache:
        _prog_cache[key] = build_program(layout)
    return _prog_cache[key]


# --------------------------------------------------------------------------
# Entry point
# --------------------------------------------------------------------------

def kernel(features, self_idx, neigh_idx, W, b, gamma, beta):
    features = np.ascontiguousarray(np.asarray(features, dtype=np.float32))
    self_idx = np.asarray(self_idx).astype(np.int64)
    neigh_idx = np.asarray(neigh_idx).astype(np.int64)
    W = np.asarray(W, dtype=np.float32)
    n, s = neigh_idx.shape
    table_rows, d = features.shape
    rpc = n // N_CORES

    # bf16 table with zero separators: [z C0 z C1 z ... C6 z]
    btab_rows = 1 + K_CHUNK * (CH + 1)
    btab = np.zeros((btab_rows, d), dtype=ml_dtypes.bfloat16)
    fb = features.astype(ml_dtypes.bfloat16)
    for c in range(K_CHUNK):
        r0 = c * CH
        r1 = min(table_rows, (c + 1) * CH)
        if r1 > r0:
            btab[1 + c * (CH + 1):1 + c * (CH + 1) + (r1 - r0)] = fb[r0:r1]

    w1t = np.ascontiguousarray(W[:, :d].T)
    w2ts = np.ascontiguousarray((W[:, d:] / float(s)).T)
    bvec = np.asarray(b, dtype=np.float32).reshape(d, 1).copy()
    gvec = np.asarray(gamma, dtype=np.float32).reshape(d, 1).copy()
    betav = np.asarray(beta, dtype=np.float32).reshape(d, 1).copy()

    layout, idx_slabs, perms = _build_layout(self_idx, neigh_idx)
    nc = _get_program(layout)

    in_maps = []
    for c in range(N_CORES):
        in_maps.append({
            "btab": btab,
            "idxslab": idx_slabs[c],
            "w1t": w1t,
            "w2ts": w2ts,
            "bvec": bvec,
            "gvec": gvec,
            "betav": betav,
        })

    global _last_in_maps
    _last_in_maps = in_maps
    res = run_bass_kernel_spmd(nc, in_maps, core_ids=list(range(N_CORES)))
    outp = np.empty((n, d), dtype=np.float32)
    for c in range(N_CORES):
        oc = res.results[c]["out"]
        outp[c * rpc:(c + 1) * rpc] = oc[perms[c]]
    return outp


_last_in_maps = None


# revision 21
# speedup vs baseline: 1.8767x; 1.1991x over previous
"""GraphSAGE layer on 8 Trainium2 NeuronCores (Bass/Tile).

Strategy: data-parallel over the 50000 target nodes (6250 rows/core),
feature table replicated per core as a bf16 copy with zero-row separators
between 7 chunks of <=28572 rows (so chunk-local row ids fit int16).
Neighbor + self rows are gathered with batched SWDGE dma_gather (one
instruction per (group, chunk, slot-batch), ~0.34ns/descriptor) instead of
per-128-row INDIRECT1D (~1.1us each). Per 4-tile group, each chunk owns a
quota of gather slots; unused positions carry idx=-1, which the ucode
resolves to (chunk base - 1 row) = a zero separator row, so dummy slots
contribute exact zeros to the slot-fold tree. Targets are greedily
grouped to minimize per-group chunk quotas. Per tile: bf16 in-place fold
tree over slots -> total sum; small tree over the 7 self slots -> self
row; agg = total - self. Then (as before) PE transposes + matmuls
(out.T = W1 @ self.T + (W2/25) @ agg.T), ReLU+bias with fused BN-stat
accumulation, AllReduce of (sum, sumsq), BN apply + row L2-normalize,
per-shard output written back and un-permuted on host.
"""
from contextlib import ExitStack

import numpy as np
import ml_dtypes

import concourse.bacc as bacc
import concourse.bass as bass
import concourse.tile as tile
from concourse import mybir
from concourse.bass_utils import run_bass_kernel_spmd
from concourse.masks import make_identity

BN_EPS = 1e-5
NORM_EPS = 1e-6

N_CORES = 8
N_TOTAL = 50000
TABLE_ROWS = 200000
D = 128
S_NEIGH = 25
P = 128
G = 2                    # tiles per group
K_CHUNK = 7
CH = 28572               # chunk rows (< 32768 for int16)
MAX_SLOT_BATCH = 14      # slots per gather instruction (num_idxs <= 7168)
S_CAP = 48               # max slots per group (SBUF: gbuf = S*G*256B/part)

_prog_cache = {}


# --------------------------------------------------------------------------
# Host-side layout construction
# --------------------------------------------------------------------------

def _greedy_group_joint(counts_all, caps):
    """Jointly assign each core's targets to groups against SHARED quota
    vectors, minimizing sum_c max(count_c) per group (max taken over the
    union of all cores' members of that group).

    counts_all: [n_cores, n, K]. caps: per-group per-core capacity.
    Returns assign[n_cores, n], pos[n_cores, n], gmax[ngroups, K].
    """
    n_cores, n, _ = counts_all.shape
    ngroups = len(caps)
    caps = np.asarray(caps, dtype=np.int64)
    gmax = np.zeros((ngroups, K_CHUNK), dtype=np.int64)
    fill = np.zeros((n_cores, ngroups), dtype=np.int64)
    assign = np.empty((n_cores, n), dtype=np.int64)
    pos = np.empty((n_cores, n), dtype=np.int64)
    slot_budget = S_CAP - K_CHUNK     # neighbor slots only (self = +7)
    big = np.iinfo(np.int64).max
    # interleave cores, targets in descending max-count order per core
    orders = [np.argsort(-counts_all[k].max(axis=1), kind="stable")
              for k in range(n_cores)]
    for j in range(n):
        for k in range(n_cores):
            i = orders[k][j]
            newmax = np.maximum(gmax, counts_all[k][i])
            delta = newmax.sum(axis=1) - gmax.sum(axis=1)
            bad = ((fill[k] >= caps)
                   | (newmax.sum(axis=1) > slot_budget))
            if bad.all():
                delta = np.where(fill[k] < caps, delta, big)
            else:
                delta = np.where(bad, big, delta)
            g = int(np.argmin(delta))
            assign[k][i] = g
            pos[k][i] = fill[k][g]
            fill[k][g] += 1
            gmax[g] = np.maximum(gmax[g], counts_all[k][i])
    return assign, pos, gmax


def _build_layout(self_idx, neigh_idx):
    """Compute the shared (across cores) slot layout + per-core idx slabs.

    Returns (layout, idx_slabs, perms):
      layout: dict with n_groups, quotas n[g][c], slot starts a[g][c], S[g],
        per-instruction list per group: (dst_slot0, nslots, col0, npos,
        nvalid), total idx columns.
      idx_slabs: [n_cores] arrays [128, totcols] int16.
      perms: [n_cores] arrays row_of_target[i] (row index in the core's
        out tensor for local target i).
    """
    n = self_idx.shape[0]
    rpc = n // N_CORES
    n_tiles = (rpc + P - 1) // P
    n_groups = (n_tiles + G - 1) // G
    caps = [min(G * P, max(0, rpc - g * G * P)) for g in range(n_groups)]

    nb_all = np.empty((N_CORES, rpc, S_NEIGH), dtype=np.int64)
    sf_all = np.empty((N_CORES, rpc), dtype=np.int64)
    counts_all = np.empty((N_CORES, rpc, K_CHUNK), dtype=np.int64)
    for k in range(N_CORES):
        sl = slice(k * rpc, (k + 1) * rpc)
        nb_all[k] = np.asarray(neigh_idx[sl], dtype=np.int64)
        sf_all[k] = np.asarray(self_idx[sl], dtype=np.int64)
        ch = nb_all[k] // CH
        counts_all[k] = (
            ch[:, :, None] == np.arange(K_CHUNK)[None, None, :]).sum(1)
    assign_all, pos_all, nq = _greedy_group_joint(counts_all, caps)
    per_core = [(nb_all[k], sf_all[k], assign_all[k], pos_all[k])
                for k in range(N_CORES)]

    # slot layout per group: chunk c -> [a_c, a_c + n_c) neighbors,
    # a_c + n_c = self slot; chunk 6 gets one extra always-zero pad slot
    # (pairs with the 7th self slot in the self fold tree).
    a = np.zeros((n_groups, K_CHUNK), dtype=np.int64)
    S = np.zeros(n_groups, dtype=np.int64)
    for g in range(n_groups):
        acc = 0
        for c in range(K_CHUNK):
            a[g, c] = acc
            acc += nq[g, c] + 1 + (1 if c == K_CHUNK - 1 else 0)
        S[g] = acc

    # per-group instruction list (split big chunks into slot batches)
    instrs = []           # per group: list of (c, slot0, nslots, col0, npos)
    gcol0 = []            # first column of each group's slab region
    col = 0
    for g in range(n_groups):
        gi = []
        gcol0.append(col)
        for c in range(K_CHUNK):
            total_slots = nq[g, c] + 1 + (1 if c == K_CHUNK - 1 else 0)
            s0 = a[g, c]
            while total_slots > 0:
                k = min(MAX_SLOT_BATCH, total_slots)
                npos = k * G * P
                gi.append((c, int(s0), int(k), int(col), int(npos)))
                col += npos // 16
                s0 += k
                total_slots -= k
        instrs.append(gi)
    totcols = col

    # per-core slabs + valid counts
    slabs = np.full((N_CORES, 16, totcols), -1, dtype=np.int16)
    valid = np.zeros((N_CORES, sum(len(gi) for gi in instrs)), dtype=np.int64)
    perms = []
    ii_of = {}
    ii = 0
    for g in range(n_groups):
        for j, ins in enumerate(instrs[g]):
            ii_of[(g, j)] = ii
            ii += 1

    for k in range(N_CORES):
        nb, sf, assign, pos = per_core[k]
        row_of = np.empty(rpc, dtype=np.int64)
        # position array per group: A[s, t, p]
        for g in range(n_groups):
            m = np.where(assign == g)[0]
            A = np.full((int(S[g]), G, P), -1, dtype=np.int64)
            if len(m) > 0:
                t_of = pos[m] // P
                p_of = pos[m] % P
                row_of[m] = (g * G + t_of) * P + p_of
                srt = np.sort(nb[m], axis=1)
                chs = srt // CH
                loc = srt - chs * CH
                # rank within chunk: count of earlier entries in same chunk
                rank = (np.arange(S_NEIGH)[None, :]
                        - (srt[:, None, :]
                           < (chs * CH)[:, :, None]).sum(-1))
                slot = a[g][chs] + rank
                A[slot.reshape(-1),
                  np.repeat(t_of, S_NEIGH),
                  np.repeat(p_of, S_NEIGH)] = loc.reshape(-1)
                sc = sf[m] // CH
                A[(a[g] + nq[g])[sc], t_of, p_of] = sf[m] - sc * CH
            for j, (c, s0, nsl, col0, npos) in enumerate(instrs[g]):
                blk = A[s0:s0 + nsl].reshape(-1).copy()
                # all positions must be valid for multi-packet gathers:
                # dummies read the trailing zero-separator row (idx CH)
                blk[blk < 0] = CH
                valid[k, ii_of[(g, j)]] = npos
                slabs[k, :, col0:col0 + npos // 16] = (
                    blk.reshape(-1, 16).T.astype(np.int16))
        perms.append(row_of)

    vmax = valid.max(axis=0)

    layout = {
        "n_groups": n_groups,
        "n_tiles": n_tiles,
        "rpc": rpc,
        "S": tuple(int(x) for x in S),
        "instrs": tuple(tuple(gi) for gi in instrs),
        "gcol0": tuple(gcol0),
        "totcols": totcols,
        "vmax": tuple(int(x) for x in vmax),
        "ii_of": ii_of,
        "a": tuple(tuple(int(x) for x in a[g]) for g in range(n_groups)),
        "nq": tuple(tuple(int(x) for x in nq[g]) for g in range(n_groups)),
    }
    idx_slabs = [np.tile(slabs[k], (8, 1)) for k in range(N_CORES)]
    return layout, idx_slabs, perms


# --------------------------------------------------------------------------
# Device program
# --------------------------------------------------------------------------

def build_program(layout):
    n_groups = layout["n_groups"]
    n_tiles = layout["n_tiles"]
    rpc = layout["rpc"]
    S = layout["S"]
    instrs = layout["instrs"]
    gcol0 = layout["gcol0"]
    totcols = layout["totcols"]
    vmax = layout["vmax"]
    ii_of = layout["ii_of"]
    maxS = max(S)
    btab_rows = 1 + K_CHUNK * (CH + 1)

    nc = bacc.Bacc("TRN2", target_bir_lowering=False, num_devices=N_CORES,
                   num_swdge_queues=4, dynamic_dma_scratch_size=32768)
    f32 = mybir.dt.float32
    bf16 = mybir.dt.bfloat16
    btab = nc.dram_tensor("btab", [btab_rows, D], bf16, kind="ExternalInput")
    idxslab = nc.dram_tensor("idxslab", [P, totcols], mybir.dt.int16,
                             kind="ExternalInput")
    w1t = nc.dram_tensor("w1t", [D, D], f32, kind="ExternalInput")
    w2ts = nc.dram_tensor("w2ts", [D, D], f32, kind="ExternalInput")
    bvec = nc.dram_tensor("bvec", [D, 1], f32, kind="ExternalInput")
    gvec = nc.dram_tensor("gvec", [D, 1], f32, kind="ExternalInput")
    betav = nc.dram_tensor("betav", [D, 1], f32, kind="ExternalInput")
    out = nc.dram_tensor("out", [rpc, D], f32, kind="ExternalOutput")

    ar_in = nc.dram_tensor("ar_in", [D, 2], f32)
    ar_out = nc.dram_tensor("ar_out", [D, 2], f32, addr_space="Shared")

    with tile.TileContext(nc) as tc:
        with ExitStack() as ctx:
            singles = ctx.enter_context(tc.tile_pool(name="singles", bufs=1))
            gpool = ctx.enter_context(tc.tile_pool(name="gpool", bufs=3))
            ipool = ctx.enter_context(tc.tile_pool(name="ipool", bufs=4))
            wpool = ctx.enter_context(tc.tile_pool(name="wpool", bufs=3))
            psum = ctx.enter_context(tc.tile_pool(name="psum", bufs=2,
                                                  space="PSUM"))
            psum2 = ctx.enter_context(tc.tile_pool(name="psum2", bufs=2,
                                                   space="PSUM"))

            w1t_sb = singles.tile([D, D], f32)
            nc.sync.dma_start(out=w1t_sb[:], in_=w1t[:])
            w2ts_sb = singles.tile([D, D], f32)
            nc.sync.dma_start(out=w2ts_sb[:], in_=w2ts[:])
            b_sb = singles.tile([D, 1], f32)
            nc.sync.dma_start(out=b_sb[:], in_=bvec[:])
            g_sb = singles.tile([D, 1], f32)
            nc.sync.dma_start(out=g_sb[:], in_=gvec[:])
            beta_sb = singles.tile([D, 1], f32)
            nc.sync.dma_start(out=beta_sb[:], in_=betav[:])
            ident = singles.tile([P, P], f32)
            make_identity(nc, ident[:])

            zbuf = singles.tile([P, n_tiles, P], f32)
            sums = singles.tile([P, n_tiles], f32)
            sumsq = singles.tile([P, n_tiles], f32)

            maxcols = max(
                sum(npos // 16 for (_, _, _, _, npos) in instrs[g])
                for g in range(n_groups))

            # ---------------- Phase A ------------------------------------
            for g in range(n_groups):
                Sg = S[g]
                cols_g = sum(npos // 16 for (_, _, _, _, npos) in instrs[g])
                idxt = ipool.tile([P, maxcols], mybir.dt.int16, tag="idxt")
                nc.sync.dma_start(
                    out=idxt[:, 0:cols_g],
                    in_=idxslab[:, gcol0[g]:gcol0[g] + cols_g])

                gbuf = gpool.tile([P, maxS * G, D], bf16, tag="gbuf")
                for j, (c, s0, nsl, col0, npos) in enumerate(instrs[g]):
                    base_c = 1 + c * (CH + 1)
                    lc0 = col0 - gcol0[g]
                    nc.gpsimd.dma_gather(
                        out_ap=gbuf[:, s0 * G:(s0 + nsl) * G, :],
                        in_ap=btab[base_c:base_c + CH + 1, :],
                        idxs_ap=idxt[:, lc0:lc0 + npos // 16],
                        num_idxs=npos,
                        num_idxs_reg=vmax[ii_of[(g, j)]],
                        elem_size=D,
                        single_packet=False,
                        queue_num=j % 4)

                gview = gbuf[:].rearrange("p (s g) d -> p s g d", g=G)
                # self slot per chunk = a_c + n_c; chunk 6 has an
                # always-zero pad slot right after its self slot
                self_slots = [layout["a"][g][c] + layout["nq"][g][c]
                              for c in range(K_CHUNK)]
                pad_slot = self_slots[K_CHUNK - 1] + 1

                for t in range(G):
                    tg = g * G + t
                    if tg >= n_tiles:
                        continue
                    nv = min(P, rpc - tg * P)

                    # self sum over the 7 self slots + zero pad slot
                    st = wpool.tile([P, 4, D], f32, tag="st")
                    nc.vector.tensor_add(
                        st[:, 0, :],
                        gview[:, self_slots[0], t, :],
                        gview[:, self_slots[1], t, :])
                    nc.vector.tensor_add(
                        st[:, 1, :],
                        gview[:, self_slots[2], t, :],
                        gview[:, self_slots[3], t, :])
                    nc.vector.tensor_add(
                        st[:, 2, :],
                        gview[:, self_slots[4], t, :],
                        gview[:, self_slots[5], t, :])
                    nc.vector.tensor_add(
                        st[:, 3, :],
                        gview[:, self_slots[6], t, :],
                        gview[:, pad_slot, t, :])
                    selfsum = wpool.tile([P, D], f32, tag="selfsum")
                    nc.vector.tensor_add(st[:, 0, :], st[:, 0, :],
                                         st[:, 1, :])
                    nc.vector.tensor_add(st[:, 2, :], st[:, 2, :],
                                         st[:, 3, :])
                    nc.vector.tensor_add(selfsum[:], st[:, 0, :],
                                         st[:, 2, :])

                    # in-place bf16 fold tree over all Sg slots
                    cur = Sg
                    while cur > 2:
                        if cur % 2 == 1:
                            nc.vector.tensor_add(
                                gview[:, 0, t, :],
                                gview[:, 0, t, :],
                                gview[:, cur - 1, t, :])
                            cur -= 1
                            if cur == 2:
                                break
                        h = cur // 2
                        nc.vector.tensor_add(
                            gview[:, 0:h, t, :],
                            gview[:, 0:h, t, :],
                            gview[:, h:cur, t, :])
                        cur = h
                    sall = wpool.tile([P, D], f32, tag="sall")
                    nc.vector.tensor_add(sall[:], gview[:, 0, t, :],
                                         gview[:, 1, t, :])
                    agg = wpool.tile([P, D], f32, tag="agg")
                    nc.vector.tensor_sub(agg[:], sall[:], selfsum[:])

                    # transposes via PE
                    pT = psum.tile([P, P], f32, tag="pT")
                    nc.tensor.transpose(out=pT[:], in_=selfsum[:],
                                        identity=ident[:])
                    sT = wpool.tile([P, P], f32, tag="sT")
                    nc.scalar.copy(out=sT[:], in_=pT[:])
                    pT2 = psum.tile([P, P], f32, tag="pT2")
                    nc.tensor.transpose(out=pT2[:], in_=agg[:],
                                        identity=ident[:])
                    aT = wpool.tile([P, P], f32, tag="aT")
                    nc.scalar.copy(out=aT[:], in_=pT2[:])

                    mm = psum2.tile([P, P], f32, tag="mm")
                    nc.tensor.matmul(mm[:], w1t_sb[:], sT[:],
                                     start=True, stop=False)
                    nc.tensor.matmul(mm[:], w2ts_sb[:], aT[:],
                                     start=False, stop=True)

                    if nv == P:
                        nc.scalar.activation(
                            out=zbuf[:, tg, :], in_=mm[:],
                            func=mybir.ActivationFunctionType.Relu,
                            bias=b_sb[:], scale=1.0,
                            accum_out=sums[:, tg:tg + 1])
                        dump = wpool.tile([P, P], f32, tag="dump")
                        nc.scalar.activation(
                            out=dump[:], in_=zbuf[:, tg, :],
                            func=mybir.ActivationFunctionType.Square,
                            accum_out=sumsq[:, tg:tg + 1])
                    else:
                        nc.scalar.activation(
                            out=zbuf[:, tg, 0:nv], in_=mm[:, 0:nv],
                            func=mybir.ActivationFunctionType.Relu,
                            bias=b_sb[:], scale=1.0,
                            accum_out=sums[:, tg:tg + 1])
                        dump = wpool.tile([P, P], f32, tag="dump")
                        nc.scalar.activation(
                            out=dump[:, 0:nv], in_=zbuf[:, tg, 0:nv],
                            func=mybir.ActivationFunctionType.Square,
                            accum_out=sumsq[:, tg:tg + 1])

            # ---------------- Phase B: global BN stats -------------------
            gstat = singles.tile([P, 2], f32)
            nc.vector.tensor_reduce(out=gstat[:, 0:1], in_=sums[:],
                                    axis=mybir.AxisListType.X,
                                    op=mybir.AluOpType.add)
            nc.vector.tensor_reduce(out=gstat[:, 1:2], in_=sumsq[:],
                                    axis=mybir.AxisListType.X,
                                    op=mybir.AluOpType.add)
            nc.sync.dma_start(out=ar_in[:], in_=gstat[:])
            nc.gpsimd.collective_compute(
                "AllReduce", mybir.AluOpType.add,
                ins=[ar_in[:]],
                outs=[ar_out[:]],
                replica_groups=[list(range(N_CORES))],
            )
            gg = singles.tile([P, 2], f32)
            nc.sync.dma_start(out=gg[:], in_=ar_out[:])

            inv_n = 1.0 / float(N_TOTAL)
            mu = singles.tile([P, 1], f32)
            nc.vector.tensor_scalar_mul(mu[:], gg[:, 0:1], inv_n)
            ex2 = singles.tile([P, 1], f32)
            nc.vector.tensor_scalar_mul(ex2[:], gg[:, 1:2], inv_n)
            var = singles.tile([P, 1], f32)
            nc.vector.tensor_mul(var[:], mu[:], mu[:])
            nc.vector.tensor_sub(var[:], ex2[:], var[:])
            nc.vector.tensor_scalar_add(var[:], var[:], BN_EPS)
            std = singles.tile([P, 1], f32)
            nc.scalar.sqrt(out=std[:], in_=var[:])
            rstd = singles.tile([P, 1], f32)
            nc.vector.reciprocal(out=rstd[:], in_=std[:])
            gp = singles.tile([P, 1], f32)
            nc.vector.tensor_mul(gp[:], g_sb[:], rstd[:])
            sh = singles.tile([P, 1], f32)
            nc.vector.tensor_mul(sh[:], mu[:], gp[:])
            nc.vector.tensor_sub(sh[:], beta_sb[:], sh[:])

            # ---------------- Phase C: BN apply + L2 normalize -----------
            for t in range(n_tiles):
                nv = min(P, rpc - t * P)
                bnz = wpool.tile([P, P], f32, tag="bnz")
                nc.vector.tensor_scalar(
                    out=bnz[:], in0=zbuf[:, t, :],
                    scalar1=gp[:], scalar2=sh[:],
                    op0=mybir.AluOpType.mult, op1=mybir.AluOpType.add)
                pT3 = psum.tile([P, P], f32, tag="pT3")
                nc.tensor.transpose(out=pT3[:], in_=bnz[:], identity=ident[:])
                yT = wpool.tile([P, P], f32, tag="yT")
                nc.scalar.copy(out=yT[:], in_=pT3[:])
                ysq = wpool.tile([P, P], f32, tag="ysq")
                n2 = wpool.tile([P, 1], f32, tag="n2")
                nc.scalar.activation(
                    out=ysq[:], in_=yT[:],
                    func=mybir.ActivationFunctionType.Square,
                    accum_out=n2[:])
                nrm = wpool.tile([P, 1], f32, tag="nrm")
                nc.scalar.sqrt(out=nrm[:], in_=n2[:])
                nc.vector.tensor_scalar_add(nrm[:], nrm[:], NORM_EPS)
                rn = wpool.tile([P, 1], f32, tag="rn")
                nc.vector.reciprocal(out=rn[:], in_=nrm[:])
                y = wpool.tile([P, P], f32, tag="y")
                nc.vector.tensor_scalar_mul(y[:], yT[:], rn[:])
                nc.sync.dma_start(out=out[t * P:t * P + nv, :],
                                  in_=y[0:nv, :])

    nc.compile()
    return nc


def _get_program(layout):
    key = (layout["S"], layout["instrs"], layout["vmax"], layout["rpc"])
    if key not in _prog_cache:
        _prog_cache[key] = build_program(layout)
    return _prog_cache[key]


# --------------------------------------------------------------------------
# Entry point
# --------------------------------------------------------------------------

def kernel(features, self_idx, neigh_idx, W, b, gamma, beta):
    features = np.ascontiguousarray(np.asarray(features, dtype=np.float32))
    self_idx = np.asarray(self_idx).astype(np.int64)
    neigh_idx = np.asarray(neigh_idx).astype(np.int64)
    W = np.asarray(W, dtype=np.float32)
    n, s = neigh_idx.shape
    table_rows, d = features.shape
    rpc = n // N_CORES

    # bf16 table with zero separators: [z C0 z C1 z ... C6 z]
    btab_rows = 1 + K_CHUNK * (CH + 1)
    btab = np.zeros((btab_rows, d), dtype=ml_dtypes.bfloat16)
    fb = features.astype(ml_dtypes.bfloat16)
    for c in range(K_CHUNK):
        r0 = c * CH
        r1 = min(table_rows, (c + 1) * CH)
        if r1 > r0:
            btab[1 + c * (CH + 1):1 + c * (CH + 1) + (r1 - r0)] = fb[r0:r1]

    w1t = np.ascontiguousarray(W[:, :d].T)
    w2ts = np.ascontiguousarray((W[:, d:] / float(s)).T)
    bvec = np.asarray(b, dtype=np.float32).reshape(d, 1).copy()
    gvec = np.asarray(gamma, dtype=np.float32).reshape(d, 1).copy()
    betav = np.asarray(beta, dtype=np.float32).reshape(d, 1).copy()

    layout, idx_slabs, perms = _build_layout(self_idx, neigh_idx)
    nc = _get_program(layout)

    in_maps = []
    for c in range(N_CORES):
        in_maps.append({
            "btab": btab,
            "idxslab": idx_slabs[c],
            "w1t": w1t,
            "w2ts": w2ts,
            "bvec": bvec,
            "gvec": gvec,
            "betav": betav,
        })

    global _last_in_maps
    _last_in_maps = in_maps
    res = run_bass_kernel_spmd(nc, in_maps, core_ids=list(range(N_CORES)))
    outp = np.empty((n, d), dtype=np.float32)
    for c in range(N_CORES):
        oc = res.results[c]["out"]
        outp[c * rpc:(c + 1) * rpc] = oc[perms[c]]
    return outp


_last_in_maps = None


# revision 22
# speedup vs baseline: 1.9218x; 1.0240x over previous
"""GraphSAGE layer on 8 Trainium2 NeuronCores (Bass/Tile).

Strategy: data-parallel over the 50000 target nodes (6250 rows/core),
feature table replicated per core as a bf16 copy with zero-row separators
between 7 chunks of <=28572 rows (so chunk-local row ids fit int16).
Neighbor + self rows are gathered with batched SWDGE dma_gather (one
instruction per (group, chunk, slot-batch), ~0.34ns/descriptor) instead of
per-128-row INDIRECT1D (~1.1us each). Per 4-tile group, each chunk owns a
quota of gather slots; unused positions carry idx=-1, which the ucode
resolves to (chunk base - 1 row) = a zero separator row, so dummy slots
contribute exact zeros to the slot-fold tree. Targets are greedily
grouped to minimize per-group chunk quotas. Per tile: bf16 in-place fold
tree over slots -> total sum; small tree over the 7 self slots -> self
row; agg = total - self. Then (as before) PE transposes + matmuls
(out.T = W1 @ self.T + (W2/25) @ agg.T), ReLU+bias with fused BN-stat
accumulation, AllReduce of (sum, sumsq), BN apply + row L2-normalize,
per-shard output written back and un-permuted on host.
"""
from contextlib import ExitStack

import numpy as np
import ml_dtypes

import concourse.bacc as bacc
import concourse.bass as bass
import concourse.tile as tile
from concourse import mybir
from concourse.bass_utils import run_bass_kernel_spmd
from concourse.masks import make_identity

BN_EPS = 1e-5
NORM_EPS = 1e-6

N_CORES = 8
N_TOTAL = 50000
TABLE_ROWS = 200000
D = 128
S_NEIGH = 25
P = 128
G = 2                    # tiles per group
K_CHUNK = 7
CH = 28572               # chunk rows (< 32768 for int16)
MAX_SLOT_BATCH = 14      # slots per gather instruction (num_idxs <= 7168)
S_CAP = 48               # max slots per group (SBUF: gbuf = S*G*256B/part)

_prog_cache = {}


# --------------------------------------------------------------------------
# Host-side layout construction
# --------------------------------------------------------------------------

def _greedy_group_joint(counts_all, caps):
    """Jointly assign each core's targets to groups against SHARED quota
    vectors, minimizing sum_c max(count_c) per group (max taken over the
    union of all cores' members of that group).

    counts_all: [n_cores, n, K]. caps: per-group per-core capacity.
    Returns assign[n_cores, n], pos[n_cores, n], gmax[ngroups, K].
    """
    n_cores, n, _ = counts_all.shape
    ngroups = len(caps)
    caps = np.asarray(caps, dtype=np.int64)
    gmax = np.zeros((ngroups, K_CHUNK), dtype=np.int64)
    fill = np.zeros((n_cores, ngroups), dtype=np.int64)
    assign = np.empty((n_cores, n), dtype=np.int64)
    pos = np.empty((n_cores, n), dtype=np.int64)
    slot_budget = S_CAP - K_CHUNK     # neighbor slots only (self = +7)
    big = np.iinfo(np.int64).max
    # interleave cores, targets in descending max-count order per core
    orders = [np.argsort(-counts_all[k].max(axis=1), kind="stable")
              for k in range(n_cores)]
    for j in range(n):
        for k in range(n_cores):
            i = orders[k][j]
            newmax = np.maximum(gmax, counts_all[k][i])
            delta = newmax.sum(axis=1) - gmax.sum(axis=1)
            bad = ((fill[k] >= caps)
                   | (newmax.sum(axis=1) > slot_budget))
            if bad.all():
                delta = np.where(fill[k] < caps, delta, big)
            else:
                delta = np.where(bad, big, delta)
            g = int(np.argmin(delta))
            assign[k][i] = g
            pos[k][i] = fill[k][g]
            fill[k][g] += 1
            gmax[g] = np.maximum(gmax[g], counts_all[k][i])
    return assign, pos, gmax


def _build_layout(self_idx, neigh_idx):
    """Compute the shared (across cores) slot layout + per-core idx slabs.

    Returns (layout, idx_slabs, perms):
      layout: dict with n_groups, quotas n[g][c], slot starts a[g][c], S[g],
        per-instruction list per group: (dst_slot0, nslots, col0, npos,
        nvalid), total idx columns.
      idx_slabs: [n_cores] arrays [128, totcols] int16.
      perms: [n_cores] arrays row_of_target[i] (row index in the core's
        out tensor for local target i).
    """
    n = self_idx.shape[0]
    rpc = n // N_CORES
    n_tiles = (rpc + P - 1) // P
    n_groups = (n_tiles + G - 1) // G
    caps = [min(G * P, max(0, rpc - g * G * P)) for g in range(n_groups)]

    nb_all = np.empty((N_CORES, rpc, S_NEIGH), dtype=np.int64)
    sf_all = np.empty((N_CORES, rpc), dtype=np.int64)
    counts_all = np.empty((N_CORES, rpc, K_CHUNK), dtype=np.int64)
    for k in range(N_CORES):
        sl = slice(k * rpc, (k + 1) * rpc)
        nb_all[k] = np.asarray(neigh_idx[sl], dtype=np.int64)
        sf_all[k] = np.asarray(self_idx[sl], dtype=np.int64)
        ch = nb_all[k] // CH
        counts_all[k] = (
            ch[:, :, None] == np.arange(K_CHUNK)[None, None, :]).sum(1)
    assign_all, pos_all, nq = _greedy_group_joint(counts_all, caps)
    per_core = [(nb_all[k], sf_all[k], assign_all[k], pos_all[k])
                for k in range(N_CORES)]

    # slot layout per group: chunk c -> [a_c, a_c + n_c) neighbors,
    # a_c + n_c = self slot; chunk 6 gets one extra always-zero pad slot
    # (pairs with the 7th self slot in the self fold tree).
    a = np.zeros((n_groups, K_CHUNK), dtype=np.int64)
    S = np.zeros(n_groups, dtype=np.int64)
    for g in range(n_groups):
        acc = 0
        for c in range(K_CHUNK):
            a[g, c] = acc
            acc += nq[g, c] + 1 + (1 if c == K_CHUNK - 1 else 0)
        S[g] = acc

    # per-group instruction list (split big chunks into slot batches)
    instrs = []           # per group: list of (c, slot0, nslots, col0, npos)
    gcol0 = []            # first column of each group's slab region
    col = 0
    for g in range(n_groups):
        gi = []
        gcol0.append(col)
        for c in range(K_CHUNK):
            total_slots = nq[g, c] + 1 + (1 if c == K_CHUNK - 1 else 0)
            s0 = a[g, c]
            while total_slots > 0:
                k = min(MAX_SLOT_BATCH, total_slots)
                npos = k * G * P
                gi.append((c, int(s0), int(k), int(col), int(npos)))
                col += npos // 16
                s0 += k
                total_slots -= k
        instrs.append(gi)
    totcols = col

    # per-core slabs + valid counts
    slabs = np.full((N_CORES, 16, totcols), -1, dtype=np.int16)
    valid = np.zeros((N_CORES, sum(len(gi) for gi in instrs)), dtype=np.int64)
    perms = []
    ii_of = {}
    ii = 0
    for g in range(n_groups):
        for j, ins in enumerate(instrs[g]):
            ii_of[(g, j)] = ii
            ii += 1

    for k in range(N_CORES):
        nb, sf, assign, pos = per_core[k]
        row_of = np.empty(rpc, dtype=np.int64)
        # position array per group: A[s, t, p]
        for g in range(n_groups):
            m = np.where(assign == g)[0]
            A = np.full((int(S[g]), G, P), -1, dtype=np.int64)
            if len(m) > 0:
                t_of = pos[m] // P
                p_of = pos[m] % P
                row_of[m] = (g * G + t_of) * P + p_of
                srt = np.sort(nb[m], axis=1)
                chs = srt // CH
                loc = srt - chs * CH
                # rank within chunk: count of earlier entries in same chunk
                rank = (np.arange(S_NEIGH)[None, :]
                        - (srt[:, None, :]
                           < (chs * CH)[:, :, None]).sum(-1))
                slot = a[g][chs] + rank
                A[slot.reshape(-1),
                  np.repeat(t_of, S_NEIGH),
                  np.repeat(p_of, S_NEIGH)] = loc.reshape(-1)
                sc = sf[m] // CH
                A[(a[g] + nq[g])[sc], t_of, p_of] = sf[m] - sc * CH
            for j, (c, s0, nsl, col0, npos) in enumerate(instrs[g]):
                blk = A[s0:s0 + nsl].reshape(-1).copy()
                # all positions must be valid for multi-packet gathers:
                # dummies read the trailing zero-separator row (idx CH)
                blk[blk < 0] = CH
                valid[k, ii_of[(g, j)]] = npos
                slabs[k, :, col0:col0 + npos // 16] = (
                    blk.reshape(-1, 16).T.astype(np.int16))
        perms.append(row_of)

    vmax = valid.max(axis=0)

    layout = {
        "n_groups": n_groups,
        "n_tiles": n_tiles,
        "rpc": rpc,
        "S": tuple(int(x) for x in S),
        "instrs": tuple(tuple(gi) for gi in instrs),
        "gcol0": tuple(gcol0),
        "totcols": totcols,
        "vmax": tuple(int(x) for x in vmax),
        "ii_of": ii_of,
        "a": tuple(tuple(int(x) for x in a[g]) for g in range(n_groups)),
        "nq": tuple(tuple(int(x) for x in nq[g]) for g in range(n_groups)),
    }
    idx_slabs = [np.tile(slabs[k], (8, 1)) for k in range(N_CORES)]
    return layout, idx_slabs, perms


# --------------------------------------------------------------------------
# Device program
# --------------------------------------------------------------------------

def build_program(layout):
    n_groups = layout["n_groups"]
    n_tiles = layout["n_tiles"]
    rpc = layout["rpc"]
    S = layout["S"]
    instrs = layout["instrs"]
    gcol0 = layout["gcol0"]
    totcols = layout["totcols"]
    vmax = layout["vmax"]
    ii_of = layout["ii_of"]
    maxS = max(S)
    btab_rows = 1 + K_CHUNK * (CH + 1)

    nc = bacc.Bacc("TRN2", target_bir_lowering=False, num_devices=N_CORES,
                   num_swdge_queues=4, dynamic_dma_scratch_size=49152)
    f32 = mybir.dt.float32
    bf16 = mybir.dt.bfloat16
    btab = nc.dram_tensor("btab", [btab_rows, D], bf16, kind="ExternalInput")
    idxslab = nc.dram_tensor("idxslab", [P, totcols], mybir.dt.int16,
                             kind="ExternalInput")
    w1t = nc.dram_tensor("w1t", [D, D], f32, kind="ExternalInput")
    w2ts = nc.dram_tensor("w2ts", [D, D], f32, kind="ExternalInput")
    bvec = nc.dram_tensor("bvec", [D, 1], f32, kind="ExternalInput")
    gvec = nc.dram_tensor("gvec", [D, 1], f32, kind="ExternalInput")
    betav = nc.dram_tensor("betav", [D, 1], f32, kind="ExternalInput")
    out = nc.dram_tensor("out", [rpc, D], f32, kind="ExternalOutput")

    ar_in = nc.dram_tensor("ar_in", [D, 2], f32)
    ar_out = nc.dram_tensor("ar_out", [D, 2], f32, addr_space="Shared")

    with tile.TileContext(nc) as tc:
        with ExitStack() as ctx:
            singles = ctx.enter_context(tc.tile_pool(name="singles", bufs=1))
            gpool = ctx.enter_context(tc.tile_pool(name="gpool", bufs=3))
            ipool = ctx.enter_context(tc.tile_pool(name="ipool", bufs=4))
            wpool = ctx.enter_context(tc.tile_pool(name="wpool", bufs=3))
            psum = ctx.enter_context(tc.tile_pool(name="psum", bufs=2,
                                                  space="PSUM"))
            psum2 = ctx.enter_context(tc.tile_pool(name="psum2", bufs=2,
                                                   space="PSUM"))

            w1t_sb = singles.tile([D, D], f32)
            nc.sync.dma_start(out=w1t_sb[:], in_=w1t[:])
            w2ts_sb = singles.tile([D, D], f32)
            nc.sync.dma_start(out=w2ts_sb[:], in_=w2ts[:])
            b_sb = singles.tile([D, 1], f32)
            nc.sync.dma_start(out=b_sb[:], in_=bvec[:])
            g_sb = singles.tile([D, 1], f32)
            nc.sync.dma_start(out=g_sb[:], in_=gvec[:])
            beta_sb = singles.tile([D, 1], f32)
            nc.sync.dma_start(out=beta_sb[:], in_=betav[:])
            ident = singles.tile([P, P], f32)
            make_identity(nc, ident[:])

            zbuf = singles.tile([P, n_tiles, P], f32)
            sums = singles.tile([P, n_tiles], f32)
            sumsq = singles.tile([P, n_tiles], f32)

            maxcols = max(
                sum(npos // 16 for (_, _, _, _, npos) in instrs[g])
                for g in range(n_groups))

            # ---------------- Phase A ------------------------------------
            for g in range(n_groups):
                Sg = S[g]
                cols_g = sum(npos // 16 for (_, _, _, _, npos) in instrs[g])
                idxt = ipool.tile([P, maxcols], mybir.dt.int16, tag="idxt")
                nc.sync.dma_start(
                    out=idxt[:, 0:cols_g],
                    in_=idxslab[:, gcol0[g]:gcol0[g] + cols_g])

                gbuf = gpool.tile([P, maxS * G, D], bf16, tag="gbuf")
                for j, (c, s0, nsl, col0, npos) in enumerate(instrs[g]):
                    base_c = 1 + c * (CH + 1)
                    lc0 = col0 - gcol0[g]
                    nc.gpsimd.dma_gather(
                        out_ap=gbuf[:, s0 * G:(s0 + nsl) * G, :],
                        in_ap=btab[base_c:base_c + CH + 1, :],
                        idxs_ap=idxt[:, lc0:lc0 + npos // 16],
                        num_idxs=npos,
                        num_idxs_reg=vmax[ii_of[(g, j)]],
                        elem_size=D,
                        single_packet=False,
                        queue_num=j % 4)

                gview = gbuf[:].rearrange("p (s g) d -> p s g d", g=G)
                # self slot per chunk = a_c + n_c; chunk 6 has an
                # always-zero pad slot right after its self slot
                self_slots = [layout["a"][g][c] + layout["nq"][g][c]
                              for c in range(K_CHUNK)]
                pad_slot = self_slots[K_CHUNK - 1] + 1

                for t in range(G):
                    tg = g * G + t
                    if tg >= n_tiles:
                        continue
                    nv = min(P, rpc - tg * P)

                    # self sum over the 7 self slots + zero pad slot
                    st = wpool.tile([P, 4, D], f32, tag="st")
                    nc.vector.tensor_add(
                        st[:, 0, :],
                        gview[:, self_slots[0], t, :],
                        gview[:, self_slots[1], t, :])
                    nc.vector.tensor_add(
                        st[:, 1, :],
                        gview[:, self_slots[2], t, :],
                        gview[:, self_slots[3], t, :])
                    nc.vector.tensor_add(
                        st[:, 2, :],
                        gview[:, self_slots[4], t, :],
                        gview[:, self_slots[5], t, :])
                    nc.vector.tensor_add(
                        st[:, 3, :],
                        gview[:, self_slots[6], t, :],
                        gview[:, pad_slot, t, :])
                    selfsum = wpool.tile([P, D], f32, tag="selfsum")
                    nc.vector.tensor_add(st[:, 0, :], st[:, 0, :],
                                         st[:, 1, :])
                    nc.vector.tensor_add(st[:, 2, :], st[:, 2, :],
                                         st[:, 3, :])
                    nc.vector.tensor_add(selfsum[:], st[:, 0, :],
                                         st[:, 2, :])

                    # in-place bf16 fold tree over all Sg slots
                    cur = Sg
                    while cur > 2:
                        if cur % 2 == 1:
                            nc.vector.tensor_add(
                                gview[:, 0, t, :],
                                gview[:, 0, t, :],
                                gview[:, cur - 1, t, :])
                            cur -= 1
                            if cur == 2:
                                break
                        h = cur // 2
                        nc.vector.tensor_add(
                            gview[:, 0:h, t, :],
                            gview[:, 0:h, t, :],
                            gview[:, h:cur, t, :])
                        cur = h
                    sall = wpool.tile([P, D], f32, tag="sall")
                    nc.vector.tensor_add(sall[:], gview[:, 0, t, :],
                                         gview[:, 1, t, :])
                    agg = wpool.tile([P, D], f32, tag="agg")
                    nc.vector.tensor_sub(agg[:], sall[:], selfsum[:])

                    # transposes via PE
                    pT = psum.tile([P, P], f32, tag="pT")
                    nc.tensor.transpose(out=pT[:], in_=selfsum[:],
                                        identity=ident[:])
                    sT = wpool.tile([P, P], f32, tag="sT")
                    nc.scalar.copy(out=sT[:], in_=pT[:])
                    pT2 = psum.tile([P, P], f32, tag="pT2")
                    nc.tensor.transpose(out=pT2[:], in_=agg[:],
                                        identity=ident[:])
                    aT = wpool.tile([P, P], f32, tag="aT")
                    nc.scalar.copy(out=aT[:], in_=pT2[:])

                    mm = psum2.tile([P, P], f32, tag="mm")
                    nc.tensor.matmul(mm[:], w1t_sb[:], sT[:],
                                     start=True, stop=False)
                    nc.tensor.matmul(mm[:], w2ts_sb[:], aT[:],
                                     start=False, stop=True)

                    if nv == P:
                        nc.scalar.activation(
                            out=zbuf[:, tg, :], in_=mm[:],
                            func=mybir.ActivationFunctionType.Relu,
                            bias=b_sb[:], scale=1.0,
                            accum_out=sums[:, tg:tg + 1])
                        dump = wpool.tile([P, P], f32, tag="dump")
                        nc.scalar.activation(
                            out=dump[:], in_=zbuf[:, tg, :],
                            func=mybir.ActivationFunctionType.Square,
                            accum_out=sumsq[:, tg:tg + 1])
                    else:
                        nc.scalar.activation(
                            out=zbuf[:, tg, 0:nv], in_=mm[:, 0:nv],
                            func=mybir.ActivationFunctionType.Relu,
                            bias=b_sb[:], scale=1.0,
                            accum_out=sums[:, tg:tg + 1])
                        dump = wpool.tile([P, P], f32, tag="dump")
                        nc.scalar.activation(
                            out=dump[:, 0:nv], in_=zbuf[:, tg, 0:nv],
                            func=mybir.ActivationFunctionType.Square,
                            accum_out=sumsq[:, tg:tg + 1])

            # ---------------- Phase B: global BN stats -------------------
            gstat = singles.tile([P, 2], f32)
            nc.vector.tensor_reduce(out=gstat[:, 0:1], in_=sums[:],
                                    axis=mybir.AxisListType.X,
                                    op=mybir.AluOpType.add)
            nc.vector.tensor_reduce(out=gstat[:, 1:2], in_=sumsq[:],
                                    axis=mybir.AxisListType.X,
                                    op=mybir.AluOpType.add)
            nc.sync.dma_start(out=ar_in[:], in_=gstat[:])
            nc.gpsimd.collective_compute(
                "AllReduce", mybir.AluOpType.add,
                ins=[ar_in[:]],
                outs=[ar_out[:]],
                replica_groups=[list(range(N_CORES))],
            )
            gg = singles.tile([P, 2], f32)
            nc.sync.dma_start(out=gg[:], in_=ar_out[:])

            inv_n = 1.0 / float(N_TOTAL)
            mu = singles.tile([P, 1], f32)
            nc.vector.tensor_scalar_mul(mu[:], gg[:, 0:1], inv_n)
            ex2 = singles.tile([P, 1], f32)
            nc.vector.tensor_scalar_mul(ex2[:], gg[:, 1:2], inv_n)
            var = singles.tile([P, 1], f32)
            nc.vector.tensor_mul(var[:], mu[:], mu[:])
            nc.vector.tensor_sub(var[:], ex2[:], var[:])
            nc.vector.tensor_scalar_add(var[:], var[:], BN_EPS)
            std = singles.tile([P, 1], f32)
            nc.scalar.sqrt(out=std[:], in_=var[:])
            rstd = singles.tile([P, 1], f32)
            nc.vector.reciprocal(out=rstd[:], in_=std[:])
            gp = singles.tile([P, 1], f32)
            nc.vector.tensor_mul(gp[:], g_sb[:], rstd[:])
            sh = singles.tile([P, 1], f32)
            nc.vector.tensor_mul(sh[:], mu[:], gp[:])
            nc.vector.tensor_sub(sh[:], beta_sb[:], sh[:])

            # ---------------- Phase C: BN apply + L2 normalize -----------
            for t in range(n_tiles):
                nv = min(P, rpc - t * P)
                bnz = wpool.tile([P, P], f32, tag="bnz")
                nc.vector.tensor_scalar(
                    out=bnz[:], in0=zbuf[:, t, :],
                    scalar1=gp[:], scalar2=sh[:],
                    op0=mybir.AluOpType.mult, op1=mybir.AluOpType.add)
                pT3 = psum.tile([P, P], f32, tag="pT3")
                nc.tensor.transpose(out=pT3[:], in_=bnz[:], identity=ident[:])
                yT = wpool.tile([P, P], f32, tag="yT")
                nc.scalar.copy(out=yT[:], in_=pT3[:])
                ysq = wpool.tile([P, P], f32, tag="ysq")
                n2 = wpool.tile([P, 1], f32, tag="n2")
                nc.scalar.activation(
                    out=ysq[:], in_=yT[:],
                    func=mybir.ActivationFunctionType.Square,
                    accum_out=n2[:])
                nrm = wpool.tile([P, 1], f32, tag="nrm")
                nc.scalar.sqrt(out=nrm[:], in_=n2[:])
                nc.vector.tensor_scalar_add(nrm[:], nrm[:], NORM_EPS)
                rn = wpool.tile([P, 1], f32, tag="rn")
                nc.vector.reciprocal(out=rn[:], in_=nrm[:])
                y = wpool.tile([P, P], f32, tag="y")
                nc.vector.tensor_scalar_mul(y[:], yT[:], rn[:])
                nc.sync.dma_start(out=out[t * P:t * P + nv, :],
                                  in_=y[0:nv, :])

    nc.compile()
    return nc


def _get_program(layout):
    key = (layout["S"], layout["instrs"], layout["vmax"], layout["rpc"])
    if key not in _prog_cache:
        _prog_cache[key] = build_program(layout)
    return _prog_cache[key]


# --------------------------------------------------------------------------
# Entry point
# --------------------------------------------------------------------------

def kernel(features, self_idx, neigh_idx, W, b, gamma, beta):
    features = np.ascontiguousarray(np.asarray(features, dtype=np.float32))
    self_idx = np.asarray(self_idx).astype(np.int64)
    neigh_idx = np.asarray(neigh_idx).astype(np.int64)
    W = np.asarray(W, dtype=np.float32)
    n, s = neigh_idx.shape
    table_rows, d = features.shape
    rpc = n // N_CORES

    # bf16 table with zero separators: [z C0 z C1 z ... C6 z]
    btab_rows = 1 + K_CHUNK * (CH + 1)
    btab = np.zeros((btab_rows, d), dtype=ml_dtypes.bfloat16)
    fb = features.astype(ml_dtypes.bfloat16)
    for c in range(K_CHUNK):
        r0 = c * CH
        r1 = min(table_rows, (c + 1) * CH)
        if r1 > r0:
            btab[1 + c * (CH + 1):1 + c * (CH + 1) + (r1 - r0)] = fb[r0:r1]

    w1t = np.ascontiguousarray(W[:, :d].T)
    w2ts = np.ascontiguousarray((W[:, d:] / float(s)).T)
    bvec = np.asarray(b, dtype=np.float32).reshape(d, 1).copy()
    gvec = np.asarray(gamma, dtype=np.float32).reshape(d, 1).copy()
    betav = np.asarray(beta, dtype=np.float32).reshape(d, 1).copy()

    layout, idx_slabs, perms = _build_layout(self_idx, neigh_idx)
    nc = _get_program(layout)

    in_maps = []
    for c in range(N_CORES):
        in_maps.append({
            "btab": btab,
            "idxslab": idx_slabs[c],
            "w1t": w1t,
            "w2ts": w2ts,
            "bvec": bvec,
            "gvec": gvec,
            "betav": betav,
        })

    global _last_in_maps
    _last_in_maps = in_maps
    res = run_bass_kernel_spmd(nc, in_maps, core_ids=list(range(N_CORES)))
    outp = np.empty((n, d), dtype=np.float32)
    for c in range(N_CORES):
        oc = res.results[c]["out"]
        outp[c * rpc:(c + 1) * rpc] = oc[perms[c]]
    return outp


_last_in_maps = None


# revision 23
# speedup vs baseline: 2.2217x; 1.1561x over previous
"""GraphSAGE layer on 8 Trainium2 NeuronCores (Bass/Tile).

Strategy: data-parallel over the 50000 target nodes (6250 rows/core),
feature table replicated per core as a bf16 copy with zero-row separators
between 7 chunks of <=28572 rows (so chunk-local row ids fit int16).
Neighbor + self rows are gathered with batched SWDGE dma_gather (one
instruction per (group, chunk, slot-batch), ~0.34ns/descriptor) instead of
per-128-row INDIRECT1D (~1.1us each). Per 4-tile group, each chunk owns a
quota of gather slots; unused positions carry idx=-1, which the ucode
resolves to (chunk base - 1 row) = a zero separator row, so dummy slots
contribute exact zeros to the slot-fold tree. Targets are greedily
grouped to minimize per-group chunk quotas. Per tile: bf16 in-place fold
tree over slots -> total sum; small tree over the 7 self slots -> self
row; agg = total - self. Then (as before) PE transposes + matmuls
(out.T = W1 @ self.T + (W2/25) @ agg.T), ReLU+bias with fused BN-stat
accumulation, AllReduce of (sum, sumsq), BN apply + row L2-normalize,
per-shard output written back and un-permuted on host.
"""
from contextlib import ExitStack

import numpy as np
import ml_dtypes

import concourse.bacc as bacc
import concourse.bass as bass
import concourse.tile as tile
from concourse import mybir
from concourse.bass_utils import run_bass_kernel_spmd
from concourse.masks import make_identity

BN_EPS = 1e-5
NORM_EPS = 1e-6

N_CORES = 8
N_TOTAL = 50000
TABLE_ROWS = 200000
D = 128
S_NEIGH = 25
P = 128
G = 2                    # tiles per group
K_CHUNK = 7
CH = 28572               # chunk rows (< 32768 for int16)
MAX_SLOT_BATCH = 14      # slots per gather instruction (num_idxs <= 7168)
S_CAP = 55               # max slots per group (SBUF: gbuf = S*G*256B/part)

_prog_cache = {}


# --------------------------------------------------------------------------
# Host-side layout construction
# --------------------------------------------------------------------------

def _greedy_group_joint(counts_all, caps):
    """Jointly assign each core's targets to groups against SHARED quota
    vectors, minimizing sum_c max(count_c) per group (max taken over the
    union of all cores' members of that group).

    counts_all: [n_cores, n, K]. caps: per-group per-core capacity.
    Returns assign[n_cores, n], pos[n_cores, n], gmax[ngroups, K].
    """
    n_cores, n, _ = counts_all.shape
    ngroups = len(caps)
    caps = np.asarray(caps, dtype=np.int64)
    gmax = np.zeros((ngroups, K_CHUNK), dtype=np.int64)
    fill = np.zeros((n_cores, ngroups), dtype=np.int64)
    assign = np.empty((n_cores, n), dtype=np.int64)
    pos = np.empty((n_cores, n), dtype=np.int64)
    slot_budget = S_CAP - K_CHUNK     # neighbor slots only (self = +7)
    big = np.iinfo(np.int64).max
    # interleave cores, targets in descending max-count order per core
    orders = [np.argsort(-counts_all[k].max(axis=1), kind="stable")
              for k in range(n_cores)]
    for j in range(n):
        for k in range(n_cores):
            i = orders[k][j]
            newmax = np.maximum(gmax, counts_all[k][i])
            delta = newmax.sum(axis=1) - gmax.sum(axis=1)
            bad = ((fill[k] >= caps)
                   | (newmax.sum(axis=1) > slot_budget))
            if bad.all():
                delta = np.where(fill[k] < caps, delta, big)
            else:
                delta = np.where(bad, big, delta)
            g = int(np.argmin(delta))
            assign[k][i] = g
            pos[k][i] = fill[k][g]
            fill[k][g] += 1
            gmax[g] = np.maximum(gmax[g], counts_all[k][i])
    return assign, pos, gmax


def _build_layout(self_idx, neigh_idx):
    """Compute the shared (across cores) slot layout + per-core idx slabs.

    Returns (layout, idx_slabs, perms):
      layout: dict with n_groups, quotas n[g][c], slot starts a[g][c], S[g],
        per-instruction list per group: (dst_slot0, nslots, col0, npos,
        nvalid), total idx columns.
      idx_slabs: [n_cores] arrays [128, totcols] int16.
      perms: [n_cores] arrays row_of_target[i] (row index in the core's
        out tensor for local target i).
    """
    n = self_idx.shape[0]
    rpc = n // N_CORES
    n_tiles = (rpc + P - 1) // P
    n_groups = (n_tiles + G - 1) // G
    caps = [min(G * P, max(0, rpc - g * G * P)) for g in range(n_groups)]

    nb_all = np.empty((N_CORES, rpc, S_NEIGH), dtype=np.int64)
    sf_all = np.empty((N_CORES, rpc), dtype=np.int64)
    counts_all = np.empty((N_CORES, rpc, K_CHUNK), dtype=np.int64)
    for k in range(N_CORES):
        sl = slice(k * rpc, (k + 1) * rpc)
        nb_all[k] = np.asarray(neigh_idx[sl], dtype=np.int64)
        sf_all[k] = np.asarray(self_idx[sl], dtype=np.int64)
        ch = nb_all[k] // CH
        counts_all[k] = (
            ch[:, :, None] == np.arange(K_CHUNK)[None, None, :]).sum(1)
    assign_all, pos_all, nq = _greedy_group_joint(counts_all, caps)
    per_core = [(nb_all[k], sf_all[k], assign_all[k], pos_all[k])
                for k in range(N_CORES)]

    # slot layout per group: chunk c -> [a_c, a_c + n_c) neighbors,
    # a_c + n_c = self slot; chunk 6 gets one extra always-zero pad slot
    # (pairs with the 7th self slot in the self fold tree).
    a = np.zeros((n_groups, K_CHUNK), dtype=np.int64)
    S = np.zeros(n_groups, dtype=np.int64)
    for g in range(n_groups):
        acc = 0
        for c in range(K_CHUNK):
            a[g, c] = acc
            acc += nq[g, c] + 1 + (1 if c == K_CHUNK - 1 else 0)
        S[g] = acc

    # per-group instruction list (split big chunks into slot batches)
    instrs = []           # per group: list of (c, slot0, nslots, col0, npos)
    gcol0 = []            # first column of each group's slab region
    col = 0
    for g in range(n_groups):
        gi = []
        gcol0.append(col)
        for c in range(K_CHUNK):
            total_slots = nq[g, c] + 1 + (1 if c == K_CHUNK - 1 else 0)
            s0 = a[g, c]
            while total_slots > 0:
                k = min(MAX_SLOT_BATCH, total_slots)
                npos = k * G * P
                gi.append((c, int(s0), int(k), int(col), int(npos)))
                col += npos // 16
                s0 += k
                total_slots -= k
        instrs.append(gi)
    totcols = col

    # per-core slabs + valid counts
    slabs = np.full((N_CORES, 16, totcols), -1, dtype=np.int16)
    valid = np.zeros((N_CORES, sum(len(gi) for gi in instrs)), dtype=np.int64)
    perms = []
    ii_of = {}
    ii = 0
    for g in range(n_groups):
        for j, ins in enumerate(instrs[g]):
            ii_of[(g, j)] = ii
            ii += 1

    for k in range(N_CORES):
        nb, sf, assign, pos = per_core[k]
        row_of = np.empty(rpc, dtype=np.int64)
        # position array per group: A[s, t, p]
        for g in range(n_groups):
            m = np.where(assign == g)[0]
            A = np.full((int(S[g]), G, P), -1, dtype=np.int64)
            if len(m) > 0:
                t_of = pos[m] // P
                p_of = pos[m] % P
                row_of[m] = (g * G + t_of) * P + p_of
                srt = np.sort(nb[m], axis=1)
                chs = srt // CH
                loc = srt - chs * CH
                # rank within chunk: count of earlier entries in same chunk
                rank = (np.arange(S_NEIGH)[None, :]
                        - (srt[:, None, :]
                           < (chs * CH)[:, :, None]).sum(-1))
                slot = a[g][chs] + rank
                A[slot.reshape(-1),
                  np.repeat(t_of, S_NEIGH),
                  np.repeat(p_of, S_NEIGH)] = loc.reshape(-1)
                sc = sf[m] // CH
                A[(a[g] + nq[g])[sc], t_of, p_of] = sf[m] - sc * CH
            for j, (c, s0, nsl, col0, npos) in enumerate(instrs[g]):
                blk = A[s0:s0 + nsl].reshape(-1).copy()
                # all positions must be valid for multi-packet gathers:
                # dummies read the trailing zero-separator row (idx CH)
                blk[blk < 0] = CH
                valid[k, ii_of[(g, j)]] = npos
                slabs[k, :, col0:col0 + npos // 16] = (
                    blk.reshape(-1, 16).T.astype(np.int16))
        perms.append(row_of)

    vmax = valid.max(axis=0)

    layout = {
        "n_groups": n_groups,
        "n_tiles": n_tiles,
        "rpc": rpc,
        "S": tuple(int(x) for x in S),
        "instrs": tuple(tuple(gi) for gi in instrs),
        "gcol0": tuple(gcol0),
        "totcols": totcols,
        "vmax": tuple(int(x) for x in vmax),
        "ii_of": ii_of,
        "a": tuple(tuple(int(x) for x in a[g]) for g in range(n_groups)),
        "nq": tuple(tuple(int(x) for x in nq[g]) for g in range(n_groups)),
    }
    idx_slabs = [np.tile(slabs[k], (8, 1)) for k in range(N_CORES)]
    return layout, idx_slabs, perms


# --------------------------------------------------------------------------
# Device program
# --------------------------------------------------------------------------

def build_program(layout):
    n_groups = layout["n_groups"]
    n_tiles = layout["n_tiles"]
    rpc = layout["rpc"]
    S = layout["S"]
    instrs = layout["instrs"]
    gcol0 = layout["gcol0"]
    totcols = layout["totcols"]
    vmax = layout["vmax"]
    ii_of = layout["ii_of"]
    maxS = max(S)
    btab_rows = 1 + K_CHUNK * (CH + 1)

    nc = bacc.Bacc("TRN2", target_bir_lowering=False, num_devices=N_CORES,
                   num_swdge_queues=4, dynamic_dma_scratch_size=49152)
    f32 = mybir.dt.float32
    bf16 = mybir.dt.bfloat16
    btab = nc.dram_tensor("btab", [btab_rows, D], bf16, kind="ExternalInput")
    idxslab = nc.dram_tensor("idxslab", [P, totcols], mybir.dt.int16,
                             kind="ExternalInput")
    w1t = nc.dram_tensor("w1t", [D, D], f32, kind="ExternalInput")
    w2ts = nc.dram_tensor("w2ts", [D, D], f32, kind="ExternalInput")
    bvec = nc.dram_tensor("bvec", [D, 1], f32, kind="ExternalInput")
    gvec = nc.dram_tensor("gvec", [D, 1], f32, kind="ExternalInput")
    betav = nc.dram_tensor("betav", [D, 1], f32, kind="ExternalInput")
    out = nc.dram_tensor("out", [rpc, D], f32, kind="ExternalOutput")

    ar_in = nc.dram_tensor("ar_in", [D, 2], f32)
    ar_out = nc.dram_tensor("ar_out", [D, 2], f32, addr_space="Shared")

    with tile.TileContext(nc) as tc:
        with ExitStack() as ctx:
            singles = ctx.enter_context(tc.tile_pool(name="singles", bufs=1))
            gpool = ctx.enter_context(tc.tile_pool(name="gpool", bufs=3))
            ipool = ctx.enter_context(tc.tile_pool(name="ipool", bufs=4))
            wpool = ctx.enter_context(tc.tile_pool(name="wpool", bufs=3))
            psum = ctx.enter_context(tc.tile_pool(name="psum", bufs=2,
                                                  space="PSUM"))
            psum2 = ctx.enter_context(tc.tile_pool(name="psum2", bufs=2,
                                                   space="PSUM"))

            w1t_sb = singles.tile([D, D], f32)
            nc.sync.dma_start(out=w1t_sb[:], in_=w1t[:])
            w2ts_sb = singles.tile([D, D], f32)
            nc.sync.dma_start(out=w2ts_sb[:], in_=w2ts[:])
            b_sb = singles.tile([D, 1], f32)
            nc.sync.dma_start(out=b_sb[:], in_=bvec[:])
            g_sb = singles.tile([D, 1], f32)
            nc.sync.dma_start(out=g_sb[:], in_=gvec[:])
            beta_sb = singles.tile([D, 1], f32)
            nc.sync.dma_start(out=beta_sb[:], in_=betav[:])
            ident = singles.tile([P, P], f32)
            make_identity(nc, ident[:])

            zbuf = singles.tile([P, n_tiles, P], f32)
            sums = singles.tile([P, n_tiles], f32)
            sumsq = singles.tile([P, n_tiles], f32)

            maxcols = max(
                sum(npos // 16 for (_, _, _, _, npos) in instrs[g])
                for g in range(n_groups))

            # ---------------- Phase A ------------------------------------
            for g in range(n_groups):
                Sg = S[g]
                cols_g = sum(npos // 16 for (_, _, _, _, npos) in instrs[g])
                idxt = ipool.tile([P, maxcols], mybir.dt.int16, tag="idxt")
                nc.sync.dma_start(
                    out=idxt[:, 0:cols_g],
                    in_=idxslab[:, gcol0[g]:gcol0[g] + cols_g])

                gbuf = gpool.tile([P, maxS * G, D], bf16, tag="gbuf")
                for j, (c, s0, nsl, col0, npos) in enumerate(instrs[g]):
                    qn = ii_of[(g, j)] % 4
                    base_c = 1 + c * (CH + 1)
                    lc0 = col0 - gcol0[g]
                    nc.gpsimd.dma_gather(
                        out_ap=gbuf[:, s0 * G:(s0 + nsl) * G, :],
                        in_ap=btab[base_c:base_c + CH + 1, :],
                        idxs_ap=idxt[:, lc0:lc0 + npos // 16],
                        num_idxs=npos,
                        num_idxs_reg=vmax[ii_of[(g, j)]],
                        elem_size=D,
                        single_packet=False,
                        queue_num=qn)

                gview = gbuf[:].rearrange("p (s g) d -> p s g d", g=G)
                # self slot per chunk = a_c + n_c; chunk 6 has an
                # always-zero pad slot right after its self slot
                self_slots = [layout["a"][g][c] + layout["nq"][g][c]
                              for c in range(K_CHUNK)]
                pad_slot = self_slots[K_CHUNK - 1] + 1

                for t in range(G):
                    tg = g * G + t
                    if tg >= n_tiles:
                        continue
                    nv = min(P, rpc - tg * P)

                    # self sum over the 7 self slots + zero pad slot
                    st = wpool.tile([P, 4, D], f32, tag="st")
                    nc.vector.tensor_add(
                        st[:, 0, :],
                        gview[:, self_slots[0], t, :],
                        gview[:, self_slots[1], t, :])
                    nc.vector.tensor_add(
                        st[:, 1, :],
                        gview[:, self_slots[2], t, :],
                        gview[:, self_slots[3], t, :])
                    nc.vector.tensor_add(
                        st[:, 2, :],
                        gview[:, self_slots[4], t, :],
                        gview[:, self_slots[5], t, :])
                    nc.vector.tensor_add(
                        st[:, 3, :],
                        gview[:, self_slots[6], t, :],
                        gview[:, pad_slot, t, :])
                    selfsum = wpool.tile([P, D], f32, tag="selfsum")
                    nc.vector.tensor_add(st[:, 0, :], st[:, 0, :],
                                         st[:, 1, :])
                    nc.vector.tensor_add(st[:, 2, :], st[:, 2, :],
                                         st[:, 3, :])
                    nc.vector.tensor_add(selfsum[:], st[:, 0, :],
                                         st[:, 2, :])

                    # in-place bf16 fold tree over all Sg slots
                    cur = Sg
                    while cur > 2:
                        if cur % 2 == 1:
                            nc.vector.tensor_add(
                                gview[:, 0, t, :],
                                gview[:, 0, t, :],
                                gview[:, cur - 1, t, :])
                            cur -= 1
                            if cur == 2:
                                break
                        h = cur // 2
                        nc.vector.tensor_add(
                            gview[:, 0:h, t, :],
                            gview[:, 0:h, t, :],
                            gview[:, h:cur, t, :])
                        cur = h
                    sall = wpool.tile([P, D], f32, tag="sall")
                    nc.vector.tensor_add(sall[:], gview[:, 0, t, :],
                                         gview[:, 1, t, :])
                    agg = wpool.tile([P, D], f32, tag="agg")
                    nc.vector.tensor_sub(agg[:], sall[:], selfsum[:])

                    # transposes via PE
                    pT = psum.tile([P, P], f32, tag="pT")
                    nc.tensor.transpose(out=pT[:], in_=selfsum[:],
                                        identity=ident[:])
                    sT = wpool.tile([P, P], f32, tag="sT")
                    nc.scalar.copy(out=sT[:], in_=pT[:])
                    pT2 = psum.tile([P, P], f32, tag="pT2")
                    nc.tensor.transpose(out=pT2[:], in_=agg[:],
                                        identity=ident[:])
                    aT = wpool.tile([P, P], f32, tag="aT")
                    nc.scalar.copy(out=aT[:], in_=pT2[:])

                    mm = psum2.tile([P, P], f32, tag="mm")
                    nc.tensor.matmul(mm[:], w1t_sb[:], sT[:],
                                     start=True, stop=False)
                    nc.tensor.matmul(mm[:], w2ts_sb[:], aT[:],
                                     start=False, stop=True)

                    if nv == P:
                        nc.scalar.activation(
                            out=zbuf[:, tg, :], in_=mm[:],
                            func=mybir.ActivationFunctionType.Relu,
                            bias=b_sb[:], scale=1.0,
                            accum_out=sums[:, tg:tg + 1])
                        dump = wpool.tile([P, P], f32, tag="dump")
                        nc.scalar.activation(
                            out=dump[:], in_=zbuf[:, tg, :],
                            func=mybir.ActivationFunctionType.Square,
                            accum_out=sumsq[:, tg:tg + 1])
                    else:
                        nc.scalar.activation(
                            out=zbuf[:, tg, 0:nv], in_=mm[:, 0:nv],
                            func=mybir.ActivationFunctionType.Relu,
                            bias=b_sb[:], scale=1.0,
                            accum_out=sums[:, tg:tg + 1])
                        dump = wpool.tile([P, P], f32, tag="dump")
                        nc.scalar.activation(
                            out=dump[:, 0:nv], in_=zbuf[:, tg, 0:nv],
                            func=mybir.ActivationFunctionType.Square,
                            accum_out=sumsq[:, tg:tg + 1])

            # ---------------- Phase B: global BN stats -------------------
            gstat = singles.tile([P, 2], f32)
            nc.vector.tensor_reduce(out=gstat[:, 0:1], in_=sums[:],
                                    axis=mybir.AxisListType.X,
                                    op=mybir.AluOpType.add)
            nc.vector.tensor_reduce(out=gstat[:, 1:2], in_=sumsq[:],
                                    axis=mybir.AxisListType.X,
                                    op=mybir.AluOpType.add)
            nc.sync.dma_start(out=ar_in[:], in_=gstat[:])
            nc.gpsimd.collective_compute(
                "AllReduce", mybir.AluOpType.add,
                ins=[ar_in[:]],
                outs=[ar_out[:]],
                replica_groups=[list(range(N_CORES))],
            )
            gg = singles.tile([P, 2], f32)
            nc.sync.dma_start(out=gg[:], in_=ar_out[:])

            inv_n = 1.0 / float(N_TOTAL)
            mu = singles.tile([P, 1], f32)
            nc.vector.tensor_scalar_mul(mu[:], gg[:, 0:1], inv_n)
            ex2 = singles.tile([P, 1], f32)
            nc.vector.tensor_scalar_mul(ex2[:], gg[:, 1:2], inv_n)
            var = singles.tile([P, 1], f32)
            nc.vector.tensor_mul(var[:], mu[:], mu[:])
            nc.vector.tensor_sub(var[:], ex2[:], var[:])
            nc.vector.tensor_scalar_add(var[:], var[:], BN_EPS)
            std = singles.tile([P, 1], f32)
            nc.scalar.sqrt(out=std[:], in_=var[:])
            rstd = singles.tile([P, 1], f32)
            nc.vector.reciprocal(out=rstd[:], in_=std[:])
            gp = singles.tile([P, 1], f32)
            nc.vector.tensor_mul(gp[:], g_sb[:], rstd[:])
            sh = singles.tile([P, 1], f32)
            nc.vector.tensor_mul(sh[:], mu[:], gp[:])
            nc.vector.tensor_sub(sh[:], beta_sb[:], sh[:])

            # ---------------- Phase C: BN apply + L2 normalize -----------
            for t in range(n_tiles):
                nv = min(P, rpc - t * P)
                bnz = wpool.tile([P, P], f32, tag="bnz")
                nc.vector.tensor_scalar(
                    out=bnz[:], in0=zbuf[:, t, :],
                    scalar1=gp[:], scalar2=sh[:],
                    op0=mybir.AluOpType.mult, op1=mybir.AluOpType.add)
                pT3 = psum.tile([P, P], f32, tag="pT3")
                nc.tensor.transpose(out=pT3[:], in_=bnz[:], identity=ident[:])
                yT = wpool.tile([P, P], f32, tag="yT")
                nc.scalar.copy(out=yT[:], in_=pT3[:])
                ysq = wpool.tile([P, P], f32, tag="ysq")
                n2 = wpool.tile([P, 1], f32, tag="n2")
                nc.scalar.activation(
                    out=ysq[:], in_=yT[:],
                    func=mybir.ActivationFunctionType.Square,
                    accum_out=n2[:])
                nrm = wpool.tile([P, 1], f32, tag="nrm")
                nc.scalar.sqrt(out=nrm[:], in_=n2[:])
                nc.vector.tensor_scalar_add(nrm[:], nrm[:], NORM_EPS)
                rn = wpool.tile([P, 1], f32, tag="rn")
                nc.vector.reciprocal(out=rn[:], in_=nrm[:])
                y = wpool.tile([P, P], f32, tag="y")
                nc.vector.tensor_scalar_mul(y[:], yT[:], rn[:])
                nc.sync.dma_start(out=out[t * P:t * P + nv, :],
                                  in_=y[0:nv, :])

    nc.compile()
    return nc


def _get_program(layout):
    key = (layout["S"], layout["instrs"], layout["vmax"], layout["rpc"])
    if key not in _prog_cache:
        _prog_cache[key] = build_program(layout)
    return _prog_cache[key]


# --------------------------------------------------------------------------
# Entry point
# --------------------------------------------------------------------------

def kernel(features, self_idx, neigh_idx, W, b, gamma, beta):
    features = np.ascontiguousarray(np.asarray(features, dtype=np.float32))
    self_idx = np.asarray(self_idx).astype(np.int64)
    neigh_idx = np.asarray(neigh_idx).astype(np.int64)
    W = np.asarray(W, dtype=np.float32)
    n, s = neigh_idx.shape
    table_rows, d = features.shape
    rpc = n // N_CORES

    # bf16 table with zero separators: [z C0 z C1 z ... C6 z]
    btab_rows = 1 + K_CHUNK * (CH + 1)
    btab = np.zeros((btab_rows, d), dtype=ml_dtypes.bfloat16)
    fb = features.astype(ml_dtypes.bfloat16)
    for c in range(K_CHUNK):
        r0 = c * CH
        r1 = min(table_rows, (c + 1) * CH)
        if r1 > r0:
            btab[1 + c * (CH + 1):1 + c * (CH + 1) + (r1 - r0)] = fb[r0:r1]

    w1t = np.ascontiguousarray(W[:, :d].T)
    w2ts = np.ascontiguousarray((W[:, d:] / float(s)).T)
    bvec = np.asarray(b, dtype=np.float32).reshape(d, 1).copy()
    gvec = np.asarray(gamma, dtype=np.float32).reshape(d, 1).copy()
    betav = np.asarray(beta, dtype=np.float32).reshape(d, 1).copy()

    layout, idx_slabs, perms = _build_layout(self_idx, neigh_idx)
    nc = _get_program(layout)

    in_maps = []
    for c in range(N_CORES):
        in_maps.append({
            "btab": btab,
            "idxslab": idx_slabs[c],
            "w1t": w1t,
            "w2ts": w2ts,
            "bvec": bvec,
            "gvec": gvec,
            "betav": betav,
        })

    global _last_in_maps
    _last_in_maps = in_maps
    res = run_bass_kernel_spmd(nc, in_maps, core_ids=list(range(N_CORES)))
    outp = np.empty((n, d), dtype=np.float32)
    for c in range(N_CORES):
        oc = res.results[c]["out"]
        outp[c * rpc:(c + 1) * rpc] = oc[perms[c]]
    return outp


_last_in_maps = None
